# revision 10
# baseline (speedup 1.0000x reference)
"""Trainium2 Bass kernel for nn_DeformAttn (deformable attention, patch-FFT).

Self-contained: hardcodes shapes for x [1,48,128,128], 8 NeuronCores,
y-band split (16 rows/core).

Design (per core):
- All deformable convs use dense hat-tap sampling with COLUMN-SHIFTED weight
  evaluation: for each x-shift delta, the 2D hat weights (hy*hx, both read
  from the delta-shifted offset field so they are evaluated at the OUTPUT
  pixel) premultiply the unshifted source on DVE (a slice of the work goes
  to the GpSimd/Pool engine); the x-shift-and-add runs on the PE as banded
  0/1 shift matmuls accumulating in PSUM, with the y-tap reduction riding
  the same PSUM accumulation (one matmul per ty plane).
- Layer-1 (1x1 deform, |off|<1): 3x3 taps.  Layer-2 (3x3 depthwise deform,
  |off|<2): 5x5 taps per kernel point, 7 global x-shifts.  Layer-3: 3x3 taps.
- qkv = A^T u with host-precomputed Khatri-Rao A (w2 (x) W1), bias via
  indicator/ones rows.  off2 via OW matrix on channel-major s1x.
- Hat weights are built on the UNSHIFTED offset field (one abs/relu pass)
  and then column-shifted by cheap SBUF->SBUF DMAs.
- x arrives in BOTH channel-major and x-major layouts from the host (no
  on-device input transposes).  vo leaves pixel-major; the host transposes.
- Patch FFT (8x8 circular conv): patch-PAIR block-diagonal real-DFT matmuls
  (128 partitions = 2 patches), complex pointwise on DVE, block-diag inverse.
- Everything bf16 on-chip except PSUM accumulation and LN statistics.
"""
import numpy as np
import ml_dtypes
_bf16 = np.float16
from contextlib import ExitStack

import concourse.bacc as bacc
import concourse.mybir as mybir
import concourse.tile as tile
from concourse.bass import AP
from concourse import bass_utils

dt = mybir.dt
F32 = dt.float32
BF16 = dt.float16
ALU = mybir.AluOpType
ACTF = mybir.ActivationFunctionType
AX = mybir.AxisListType

H = W = 128
BND = 16          # band rows per core
NYS = 22          # s1x rows per core (band +/-3)
NYX = 24          # x rows per core (band +/-4)
NYV = 18          # vo rows per core (band +/-1)
NCORES = 8
CB = [0, 128, 256, 314]   # uT K-chunk bases over 442 slots

# sampling premultiplies routed to the GpSimd (Pool) engine: (k, s) pairs
POOL_KS = {(k, 2) for k in range(9)}


def _v(t, off, dims):
    """View of tile t: keep its full partition dim, custom free dims."""
    return AP(t.tensor, t.offset + off, [list(t.ap[0])] + [list(d) for d in dims])


def _vp(t, p0, np_, off, dims):
    """View with partition sub-range [p0, p0+np_) and custom free dims."""
    st = t.ap[0][0]
    return AP(t.tensor, t.offset + p0 * st + off,
              [[st, np_]] + [list(d) for d in dims])


def build_kernel_a(fold_ln=False):
    nc = bacc.Bacc("TRN2", target_bir_lowering=False, debug=False)
    xh_d = nc.dram_tensor("xh", [48, NYS, W], BF16, kind="ExternalInput")
    xt_d = nc.dram_tensor("xt", [128, 48, NYX], BF16, kind="ExternalInput")
    onesp_d = nc.dram_tensor("onesp", [128, NYS], BF16, kind="ExternalInput")
    identb_d = nc.dram_tensor("identb", [128, 128], BF16, kind="ExternalInput")
    ow1T_d = nc.dram_tensor("ow1T", [48, 2], BF16, kind="ExternalInput")
    owt_d = nc.dram_tensor("owt", [49, 9, 18], BF16, kind="ExternalInput")
    a4_d = nc.dram_tensor("a4", [128, 4, 288], BF16, kind="ExternalInput")
    tf2Re_d = nc.dram_tensor("tf2Re", [128, 80], BF16, kind="ExternalInput")
    tf2Im_d = nc.dram_tensor("tf2Im", [128, 80], BF16, kind="ExternalInput")
    ti2Re_d = nc.dram_tensor("ti2Re", [80, 128], BF16, kind="ExternalInput")
    ti2Im_d = nc.dram_tensor("ti2Im", [80, 128], BF16, kind="ExternalInput")
    s7_d = nc.dram_tensor("s7", [128, 7, 128], BF16, kind="ExternalInput")
    tau3_d = nc.dram_tensor("tau3", [128, 3], BF16, kind="ExternalInput")
    tau5_d = nc.dram_tensor("tau5", [128, 5], BF16, kind="ExternalInput")
    tau79y_d = nc.dram_tensor("tau79y", [128, 7, 9, BND], BF16,
                              kind="ExternalInput")
    lnw_d = nc.dram_tensor("lnw", [128, 96], BF16, kind="ExternalInput")
    lnb_d = nc.dram_tensor("lnb", [128, 96], BF16, kind="ExternalInput")
    vo_out = nc.dram_tensor("vo_out", [128, BND, 96], BF16,
                            kind="ExternalOutput")

    with tile.TileContext(nc) as tc, ExitStack() as top:
        cpool = top.enter_context(tc.tile_pool(name="consts", bufs=1))
        xh = cpool.tile([48, NYS, W], BF16)
        xt = cpool.tile([128, 48, NYX], BF16)
        identb = cpool.tile([128, 128], BF16)
        ow1T = cpool.tile([48, 2], BF16)
        owt = cpool.tile([49, 9, 18], BF16)
        a4 = cpool.tile([128, 4, 288], BF16)
        tf2Re = cpool.tile([128, 80], BF16)
        tf2Im = cpool.tile([128, 80], BF16)
        ti2Re = cpool.tile([80, 128], BF16)
        ti2Im = cpool.tile([80, 128], BF16)
        s7 = cpool.tile([128, 7, 128], BF16)
        tau3 = cpool.tile([128, 3], BF16)
        tau5 = cpool.tile([128, 5], BF16)
        tau79y = cpool.tile([128, 7, 9, BND], BF16)
        lnw = cpool.tile([128, 96], BF16)
        lnb = cpool.tile([128, 96], BF16)
        onesp = cpool.tile([128, NYS], BF16, name="onesp")
        # ordered by first use: off1 path, layer-1 path, then the rest
        first = [(xh, xh_d), (ow1T, ow1T_d), (tau3, tau3_d), (xt, xt_d),
                 (s7, s7_d), (identb, identb_d), (onesp, onesp_d),
                 (owt, owt_d), (tau5, tau5_d), (tau79y, tau79y_d),
                 (a4, a4_d), (tf2Re, tf2Re_d), (tf2Im, tf2Im_d),
                 (ti2Re, ti2Re_d), (ti2Im, ti2Im_d)]
        if not fold_ln:
            first += [(lnw, lnw_d), (lnb, lnb_d)]
        for sb, dr in first:
            nc.sync.dma_start(sb[:], dr[:])

        psum = top.enter_context(tc.tile_pool(name="psum", bufs=3, space="PSUM"))
        pupool = top.enter_context(tc.tile_pool(name="pu", bufs=4, space="PSUM"))

        big_cm = tc.tile_pool(name="big", bufs=1)
        bpool = big_cm.__enter__()
        s1x = bpool.tile([128, 49, NYS], BF16)
        u = bpool.tile([128, 442, BND], BF16)
        uT = bpool.tile([128, 4, 16, 128], BF16)
        qkv = bpool.tile([128, 16, 288], BF16)

        # ================= phase X: off1, layer-1 ==========================
        with tc.tile_pool(name="px", bufs=1) as p1:
            # off1 on s1x rows -> one psum, region-accumulated
            po = pupool.tile([128, 512], F32, tag="acc", name="po", bufs=3)
            for y in range(NYS):
                nc.tensor.matmul(po[:128, 2 * y:2 * y + 2],
                                 _v(xh, y * W, [[1, 128]]),
                                 ow1T[:], start=True, stop=True)
            off1pm = p1.tile([128, NYS, 2], BF16)
            nc.scalar.copy(off1pm[:], po[:128, :44])
            # shifted offset copies: off1s3 [128, 3, 22, 2]
            off1s3 = p1.tile([128, 3, NYS, 2], BF16)
            nc.gpsimd.memset(off1s3[:], 0.0)
            nc.vector.tensor_copy(off1s3[:, 1], off1pm[:])
            # slot d=-1 (di=0): w[xin] = off[xin+1]; slot d=+1: off[xin-1]
            nc.sync.dma_start(off1s3[0:127, 0], off1pm[1:128])
            nc.sync.dma_start(off1s3[1:128, 2], off1pm[0:127])
            # W1d [128, 3d, 3ty, 22] = hat(oy_sh - (ty-1)) * hat(ox_sh - d)
            w1d = p1.tile([128, 3, 3, NYS], BF16)
            hx1 = p1.tile([128, 3, NYS], BF16)
            with nc.allow_low_precision(reason="hat weights bf16"):
                nc.vector.tensor_tensor(
                    out=w1d[:],
                    in0=_v(off1s3, 0, [[2 * NYS, 3], [0, 3], [2, NYS]]),
                    in1=_v(tau3, 0, [[0, 3], [1, 3], [0, NYS]]),
                    op=ALU.subtract)
                nc.scalar.activation(w1d[:], w1d[:], ACTF.Abs)
                nc.scalar.activation(w1d[:], w1d[:], ACTF.Relu,
                                     bias=1.0, scale=-1.0)
                nc.vector.tensor_tensor(
                    out=hx1[:],
                    in0=_v(off1s3, 1, [[2 * NYS, 3], [2, NYS]]),
                    in1=_v(tau3, 0, [[1, 3], [0, NYS]]),
                    op=ALU.subtract)
                nc.scalar.activation(hx1[:], hx1[:], ACTF.Abs)
                nc.scalar.activation(hx1[:], hx1[:], ACTF.Relu,
                                     bias=1.0, scale=-1.0)
                nc.vector.tensor_tensor(
                    out=w1d[:], in0=w1d[:],
                    in1=_v(hx1, 0, [[NYS, 3], [0, 3], [1, NYS]]),
                    op=ALU.mult)
            # layer-1 sampling: premult per delta, PE shift-accumulate
            ps1 = [pupool.tile([128, 512], F32, tag="acc", bufs=3, name=f"ps1_{c}")
                   for c in range(3)]
            for di in range(3):
                tmp1 = p1.tile([128, 3, 48, NYS], BF16, tag="tmp1",
                               name="tmp1", bufs=2)
                with nc.allow_low_precision(reason="sampling taps bf16"):
                    nc.vector.tensor_tensor(
                        out=tmp1[:],
                        in0=_v(xt, 0, [[1, 3], [NYX, 48], [1, NYS]]),
                        in1=_v(w1d, di * 3 * NYS, [[NYS, 3], [0, 48], [1, NYS]]),
                        op=ALU.mult)
                lhs = _v(s7, (di + 2) * 128, [[1, 128]])  # delta=-1,0,1 -> slots 2,3,4
                for ty in range(3):
                    for ch in range(3):
                        nc.tensor.matmul(
                            ps1[ch][:128, :352], lhs,
                            _v(tmp1, ty * 48 * NYS + ch * 352, [[1, 352]]),
                            start=(di == 0 and ty == 0),
                            stop=(di == 2 and ty == 2))
            for ch in range(3):
                (nc.scalar.copy if ch != 1 else nc.vector.tensor_copy)(
                    _v(s1x, ch * 352, [[1, 352]]), ps1[ch][:128, :352])
            nc.sync.dma_start(_v(s1x, 48 * NYS, [[1, NYS]]), onesp[:])

        # ================= phase O: s1xT, off2, W2d =========================
        w2d = bpool.tile([128, 7, 9, 5, BND], BF16)
        with tc.tile_pool(name="po2", bufs=1) as p2:
            hy7 = p2.tile([128, 7, 9, 5, BND], BF16)
            ox7 = p2.tile([128, 7, 9, BND], BF16)
            nc.gpsimd.memset(hy7[:], 0.0)
            nc.gpsimd.memset(ox7[:], 0.0)
            s1xT = p2.tile([49, NYS, 130], BF16)
            nc.gpsimd.memset(_v(s1xT, 0, [[130, NYS], [1, 1]]), 0.0)
            nc.gpsimd.memset(_v(s1xT, 129, [[130, NYS], [1, 1]]), 0.0)
            for gi, (g0, gn) in enumerate([(0, 8), (8, 8), (16, 6)]):
                ps = psum.tile([128, 1024], BF16, tag="psb", name="ps", bufs=3)
                for i in range(gn):
                    nc.tensor.transpose(ps[:49, i * 128:(i + 1) * 128],
                                        _v(s1x, g0 + i, [[NYS, 49]]),
                                        identb[:, :])
                dst = _v(s1xT, g0 * 130 + 1, [[130, gn], [1, 128]])
                (nc.scalar.copy if gi % 2 == 0 else nc.vector.tensor_copy)(
                    dst, ps[:49, :gn * 128])
            # off2: per band row b, 9 taps accumulate; 2 psum region-tiles
            pofs = [pupool.tile([128, 512], F32, tag="acc", bufs=3, name=f"po2_{h}")
                    for h in range(2)]
            for b in range(BND):
                po2 = pofs[b // 8]
                col = 18 * (b % 8)
                for t in range(9):
                    ty, tx = divmod(t, 3)
                    nc.tensor.matmul(
                        po2[:128, col:col + 18],
                        _v(s1xT, (b + 2 + ty) * 130 + tx, [[1, 128]]),
                        owt[:, t], start=(t == 0), stop=(t == 8))
            off2pm = p2.tile([128, BND, 18], BF16)
            nc.scalar.copy(_v(off2pm, 0, [[1, 144]]), pofs[0][:128, :144])
            nc.scalar.copy(_v(off2pm, 144, [[1, 144]]), pofs[1][:128, :144])
            # base hat_y on the UNSHIFTED oy field: hyb[k,ty,y]
            hyb = p2.tile([128, 9, 5, BND], BF16)
            with nc.allow_low_precision(reason="hat weights bf16"):
                nc.vector.tensor_tensor(
                    out=hyb[:],
                    in0=_v(off2pm, 0, [[2, 9], [0, 5], [18, BND]]),
                    in1=_v(tau5, 0, [[0, 9], [1, 5], [0, BND]]),
                    op=ALU.subtract)
                nc.scalar.activation(hyb[:], hyb[:], ACTF.Abs)
                nc.scalar.activation(hyb[:], hyb[:], ACTF.Relu,
                                     bias=1.0, scale=-1.0)
            # compact ox field, then column-shifted copies:
            # hy7[di] = hyb[x - d], ox7[di] = oxb[x - d]
            oxb = p2.tile([128, 9, BND], BF16)
            nc.vector.tensor_copy(
                oxb[:], _v(off2pm, 1, [[2, 9], [18, BND]]))
            for d in range(-3, 4):
                di = d + 3
                if d > 0:
                    nc.sync.dma_start(hy7[d:128, di], hyb[0:128 - d])
                    nc.sync.dma_start(ox7[d:128, di], oxb[0:128 - d])
                elif d < 0:
                    nc.sync.dma_start(hy7[0:128 + d, di], hyb[-d:128])
                    nc.sync.dma_start(ox7[0:128 + d, di], oxb[-d:128])
                else:
                    nc.sync.dma_start(hy7[:, di], hyb[:])
                    nc.sync.dma_start(ox7[:, di], oxb[:])
            # hxb[di,k,y] = hat(ox_sh - tau79[di,k]); w2d = hy7 * hxb
            hxb = p2.tile([128, 7, 9, BND], BF16)
            with nc.allow_low_precision(reason="hat weights bf16"):
                nc.vector.tensor_tensor(out=hxb[:], in0=ox7[:], in1=tau79y[:],
                                        op=ALU.subtract)
                nc.scalar.activation(hxb[:], hxb[:], ACTF.Abs)
                nc.scalar.activation(hxb[:], hxb[:], ACTF.Relu,
                                     bias=1.0, scale=-1.0)
                for di in range(7):
                    nc.vector.tensor_tensor(
                        out=w2d[:, di], in0=hy7[:, di],
                        in1=_v(hxb, di * 9 * BND, [[BND, 9], [0, 5], [1, BND]]),
                        op=ALU.mult)

        # ========== phase S: sampling (k loop, uT interleaved) =============
        uT_after = {2: [0], 5: [1], 7: [2], 8: [3]}

        def emit_uT_chunk(c):
            for g in range(2):
                ps = psum.tile([128, 1024], BF16, tag="psb", name="ps", bufs=3)
                for yy in range(8):
                    y = g * 8 + yy
                    nc.tensor.transpose(
                        ps[:128, yy * 128:(yy + 1) * 128],
                        _v(u, CB[c] * BND + y, [[BND, 128]]),
                        identb[:, :])
                dst = _v(uT, c * 2048 + g * 64, [[8, 8], [128, 16], [1, 8]])
                (nc.scalar.copy if (c + g) % 2 == 0 else nc.vector.tensor_copy)(
                    dst, ps[:128, :1024])

        with tc.tile_pool(name="psmp", bufs=1) as p3:
            for k in range(9):
                ki, kj = divmod(k, 3)
                pk = [pupool.tile([128, 512], F32, tag="acc", bufs=3,
                                  name=f"pk{ch}") for ch in range(2)]
                for s in range(5):
                    d = kj - 3 + s
                    di = d + 3
                    tmp = p3.tile([128, 5, 49, BND], BF16, tag="tmp",
                                  name="tmp", bufs=3)
                    eng = nc.gpsimd if (k, s) in POOL_KS else nc.vector
                    with nc.allow_low_precision(reason="sampling taps bf16"):
                        eng.tensor_tensor(
                            out=tmp[:],
                            in0=_v(s1x, ki, [[1, 5], [NYS, 49], [1, BND]]),
                            in1=_v(w2d, di * 9 * 5 * BND + k * 5 * BND,
                                   [[BND, 5], [0, 49], [1, BND]]),
                            op=ALU.mult)
                    lhs = _v(s7, di * 128, [[1, 128]])
                    for ty in range(5):
                        for ch in range(2):
                            nc.tensor.matmul(
                                pk[ch][:128, :392], lhs,
                                _v(tmp, ty * 784 + ch * 392, [[1, 392]]),
                                start=(s == 0 and ty == 0),
                                stop=(s == 4 and ty == 4))
                nc.scalar.copy(_v(u, k * 784, [[1, 392]]),
                               pk[0][:128, :392])
                nc.scalar.copy(_v(u, k * 784 + 392, [[1, 392]]),
                               pk[1][:128, :392])
                if k == 8:
                    nc.vector.memset(_v(u, 441 * BND, [[1, BND]]), 1.0)
                for c in uT_after.get(k, []):
                    emit_uT_chunk(c)

        # ================= phase Q + FFT (interleaved) ======================
        with tc.tile_pool(name="pfft", bufs=1) as fp:
            qhRe = fp.tile([80, 16, 192], BF16)
            qhIm = fp.tile([80, 16, 192], BF16)
            for g in range(8):
                for pc in (2 * g, 2 * g + 1):
                    qp = pupool.tile([128, 512], F32, tag="acc", bufs=3,
                                     name="qp")
                    for c in range(4):
                        nc.tensor.matmul(qp[:128, :288],
                                         _v(uT, c * 2048 + pc * 128, [[1, 128]]),
                                         a4[:, c], start=(c == 0), stop=(c == 3))
                    if pc % 2 == 0:
                        nc.scalar.copy(_v(qkv, pc * 288, [[1, 288]]),
                                       qp[:128, :288])
                    else:
                        nc.vector.tensor_copy(_v(qkv, pc * 288, [[1, 288]]),
                                              qp[:128, :288])
                rhs = _v(qkv, 2 * g * 288, [[288, 2], [1, 192]])
                psR = psum.tile([128, 512], F32, tag="ps", name="ps", bufs=2)
                nc.tensor.matmul(psR[:80, :384], tf2Re[:], rhs,
                                 start=True, stop=True)
                nc.scalar.copy(_v(qhRe, 2 * g * 192, [[1, 384]]),
                               psR[:80, :384])
                psI = psum.tile([128, 512], F32, tag="ps", name="ps", bufs=2)
                nc.tensor.matmul(psI[:80, :384], tf2Im[:], rhs,
                                 start=True, stop=True)
                nc.vector.tensor_copy(_v(qhIm, 2 * g * 192, [[1, 384]]),
                                      psI[:80, :384])
            ohRe = fp.tile([80, 16, 96], BF16)
            ohIm = fp.tile([80, 16, 96], BF16)
            t1 = fp.tile([80, 16, 96], BF16)
            t2 = fp.tile([80, 16, 96], BF16)
            ar = _v(qhRe, 0, [[192, 16], [1, 96]])
            br = _v(qhRe, 96, [[192, 16], [1, 96]])
            ai = _v(qhIm, 0, [[192, 16], [1, 96]])
            bi = _v(qhIm, 96, [[192, 16], [1, 96]])
            with nc.allow_low_precision(reason="fft products bf16"):
                nc.vector.tensor_tensor(out=t1[:], in0=ar, in1=br, op=ALU.mult)
                nc.gpsimd.tensor_tensor(out=t2[:], in0=ai, in1=bi, op=ALU.mult)
                nc.vector.tensor_tensor(out=ohRe[:], in0=t1[:], in1=t2[:],
                                        op=ALU.subtract)
                nc.vector.tensor_tensor(out=t1[:], in0=ar, in1=bi, op=ALU.mult)
                nc.vector.tensor_tensor(out=t2[:], in0=ai, in1=br, op=ALU.mult)
                nc.vector.tensor_tensor(out=ohIm[:], in0=t1[:], in1=t2[:],
                                        op=ALU.add)
            osb = fp.tile([128, 16, 96], BF16)
            for g in range(4):
                ps = psum.tile([128, 512], F32, tag="ps", name="ps", bufs=2)
                nc.tensor.matmul(ps[:128, :384], ti2Re[:],
                                 _v(ohRe, g * 384, [[1, 384]]),
                                 start=True, stop=False)
                nc.tensor.matmul(ps[:128, :384], ti2Im[:],
                                 _v(ohIm, g * 384, [[1, 384]]),
                                 start=False, stop=True)
                (nc.scalar.copy if g % 2 == 0 else nc.vector.tensor_copy)(
                    _v(osb, g * 384, [[1, 384]]), ps[:128, :384])
            # LayerNorm over 96 channels per pixel
            ssum = fp.tile([128, BND], F32)
            sqs = fp.tile([128, BND], F32)
            sq = fp.tile([128, 16, 96], BF16)
            mu = fp.tile([128, BND], F32)
            var = fp.tile([128, BND], F32)
            std = fp.tile([128, BND], F32)
            inv = fp.tile([128, BND], F32)
            musq = fp.tile([128, BND], F32)
            mub = fp.tile([128, BND], BF16)
            invb = fp.tile([128, BND], BF16)
            on = fp.tile([128, 16, 96], BF16)
            vo = fp.tile([128, 16, 96], BF16)
            nc.vector.tensor_reduce(out=ssum[:], in_=osb[:], axis=AX.X,
                                    op=ALU.add)
            nc.scalar.activation(sq[:], osb[:], ACTF.Square)
            nc.vector.tensor_reduce(out=sqs[:], in_=sq[:], axis=AX.X,
                                    op=ALU.add)
            nc.scalar.activation(mu[:], ssum[:], ACTF.Copy, scale=1.0 / 96)
            nc.scalar.activation(var[:], sqs[:], ACTF.Copy, scale=1.0 / 96)
            nc.scalar.activation(musq[:], mu[:], ACTF.Square)
            nc.vector.tensor_tensor(out=var[:], in0=var[:], in1=musq[:],
                                    op=ALU.subtract)
            nc.vector.tensor_scalar_add(out=var[:], in0=var[:], scalar1=1e-5)
            nc.scalar.activation(std[:], var[:], ACTF.Sqrt)
            nc.vector.reciprocal(inv[:], std[:])
            nc.vector.tensor_copy(mub[:], mu[:])
            nc.vector.tensor_copy(invb[:], inv[:])
            with nc.allow_low_precision(reason="ln bf16"):
                nc.vector.tensor_tensor(out=on[:], in0=osb[:],
                                        in1=_v(mub, 0, [[1, 16], [0, 96]]),
                                        op=ALU.subtract)
                nc.vector.tensor_tensor(out=on[:], in0=on[:],
                                        in1=_v(invb, 0, [[1, 16], [0, 96]]),
                                        op=ALU.mult)
                if not fold_ln:
                    nc.vector.tensor_tensor(out=on[:], in0=on[:],
                                            in1=_v(lnw, 0, [[0, 16], [1, 96]]),
                                            op=ALU.mult)
                    nc.vector.tensor_tensor(out=on[:], in0=on[:],
                                            in1=_v(lnb, 0, [[0, 16], [1, 96]]),
                                            op=ALU.add)
                nc.vector.tensor_tensor(out=vo[:], in0=on[:],
                                        in1=_v(qkv, 192, [[288, 16], [1, 96]]),
                                        op=ALU.mult)
            nc.sync.dma_start(vo_out[:], vo[:])
        big_cm.__exit__(None, None, None)
    nc.compile()
    return nc


def build_kernel_b():
    """Layer-3: out = W3 @ deform_sample(vo) + b3 == deform_sample(W3 @ vo) + b3
    (sampling offsets are channel-shared, so the contraction commutes).
    W3-contraction and off3 ride ONE fused per-row matmul; output leaves
    x-major (host transposes back)."""
    nc = bacc.Bacc("TRN2", target_bir_lowering=False, debug=False)
    voh_d = nc.dram_tensor("voh", [96, NYV, 128], BF16, kind="ExternalInput")
    wo50_d = nc.dram_tensor("wo50", [96, 50], BF16, kind="ExternalInput")
    b3r_d = nc.dram_tensor("b3r", [128, 48], F32, kind="ExternalInput")
    tau3_d = nc.dram_tensor("tau3", [128, 3], BF16, kind="ExternalInput")
    s3_d = nc.dram_tensor("s3m", [128, 3, 128], BF16, kind="ExternalInput")
    out_d = nc.dram_tensor("outp", [128, 48, BND], F32, kind="ExternalOutput")
    with tile.TileContext(nc) as tc, ExitStack() as top:
        pool = top.enter_context(tc.tile_pool(name="b", bufs=1))
        pup = top.enter_context(tc.tile_pool(name="pub", bufs=4, space="PSUM"))
        voh = pool.tile([96, NYV, 128], BF16)
        wo50 = pool.tile([96, 50], BF16)
        b3r = pool.tile([128, 48], F32)
        tau3 = pool.tile([128, 3], BF16)
        s3m = pool.tile([128, 3, 128], BF16)
        for sb, dr in [(wo50, wo50_d), (tau3, tau3_d), (s3m, s3_d),
                       (b3r, b3r_d)]:
            nc.sync.dma_start(sb[:], dr[:])
        for rc in range(3):
            nc.sync.dma_start(voh[:, 6 * rc:6 * (rc + 1)],
                              voh_d[:, 6 * rc:6 * (rc + 1)])
        # fused [W3 | ow3] per-row contraction -> wvx2 [128, 50, 18] c-major
        wvx2 = pool.tile([128, 50, NYV], BF16)
        pws = [pup.tile([128, 512], F32, tag="acc", bufs=4, name=f"pw{h}")
               for h in range(2)]
        for y in range(NYV):
            nc.tensor.matmul(pws[y // 9][:128, (y % 9) * 50:(y % 9) * 50 + 50],
                             _v(voh, y * 128, [[1, 128]]),
                             wo50[:], start=True, stop=True)
        for h in range(2):
            dst = _v(wvx2, h * 9, [[1, 9], [NYV, 50]])
            (nc.scalar.copy if h == 0 else nc.vector.tensor_copy)(
                dst, pws[h][:128, :450])
        # off3 values live at wvx2 cols 48:50: off3[x, o, y] = wvx2[x, 48+o, y]
        off3s3 = pool.tile([128, 3, BND, 2], BF16)
        nc.gpsimd.memset(off3s3[:], 0.0)
        # center (d=0): off3s3[:, 1, b, o] = off3[x, o, b+1]
        nc.vector.tensor_copy(
            off3s3[:, 1], _v(wvx2, 48 * NYV + 1, [[1, BND], [NYV, 2]]))
        nc.sync.dma_start(off3s3[0:127, 0], off3s3[1:128, 1])
        nc.sync.dma_start(off3s3[1:128, 2], off3s3[0:127, 1])
        w3d = pool.tile([128, 3, 3, BND], BF16)
        hx3 = pool.tile([128, 3, BND], BF16)
        with nc.allow_low_precision(reason="hat weights bf16"):
            nc.vector.tensor_tensor(
                out=w3d[:],
                in0=_v(off3s3, 0, [[2 * BND, 3], [0, 3], [2, BND]]),
                in1=_v(tau3, 0, [[0, 3], [1, 3], [0, BND]]),
                op=ALU.subtract)
            nc.scalar.activation(w3d[:], w3d[:], ACTF.Abs)
            nc.scalar.activation(w3d[:], w3d[:], ACTF.Relu, bias=1.0, scale=-1.0)
            nc.vector.tensor_tensor(
                out=hx3[:],
                in0=_v(off3s3, 1, [[2 * BND, 3], [2, BND]]),
                in1=_v(tau3, 0, [[1, 3], [0, BND]]),
                op=ALU.subtract)
            nc.scalar.activation(hx3[:], hx3[:], ACTF.Abs)
            nc.scalar.activation(hx3[:], hx3[:], ACTF.Relu, bias=1.0, scale=-1.0)
            nc.vector.tensor_tensor(
                out=w3d[:], in0=w3d[:],
                in1=_v(hx3, 0, [[BND, 3], [0, 3], [1, BND]]),
                op=ALU.mult)
        pv = [pup.tile([128, 512], F32, tag="acc", bufs=4, name=f"pv{ch}")
              for ch in range(2)]
        for di in range(3):
            tmp3 = pool.tile([128, 3, 48, BND], BF16, tag="tmp3",
                             name="tmp3", bufs=2)
            with nc.allow_low_precision(reason="sampling taps bf16"):
                nc.vector.tensor_tensor(
                    out=tmp3[:],
                    in0=_v(wvx2, 0, [[1, 3], [NYV, 48], [1, BND]]),
                    in1=_v(w3d, di * 3 * BND, [[BND, 3], [0, 48], [1, BND]]),
                    op=ALU.mult)
            lhs = _v(s3m, di * 128, [[1, 128]])
            for ty in range(3):
                for ch in range(2):
                    nc.tensor.matmul(
                        pv[ch][:128, :384], lhs,
                        _v(tmp3, ty * 48 * BND + ch * 384, [[1, 384]]),
                        start=(di == 0 and ty == 0),
                        stop=(di == 2 and ty == 2))
        # evict + bias -> x-major f32 out [128, 48, 16]
        s3of = pool.tile([128, 48, BND], F32)
        for ch in range(2):
            nc.vector.tensor_tensor(
                out=_v(s3of, ch * 384, [[1, 384]]),
                in0=pv[ch][:128, :384],
                in1=_vp(b3r, 0, 128, ch * 24, [[1, 24], [0, BND]]),
                op=ALU.add)
        nc.sync.dma_start(out_d[:], s3of[:])
    nc.compile()
    return nc


def _hat_consts(inputs):
    fold_ln = not np.any(inputs['ln_b'])
    lnscale = (inputs['ln_w'].astype(np.float32) if fold_ln
               else np.ones(96, np.float32))
    w1 = inputs['w1'][:, :, 0, 0].astype(np.float32)
    b1 = inputs['b1'].astype(np.float32)
    ow2 = inputs['off_w2'].astype(np.float32)
    w2f = inputs['w2'][:, 0].reshape(288, 9).astype(np.float32)
    OW = np.zeros((18, 9, 49), np.float32)
    for ty in range(3):
        for tx in range(3):
            t = ty * 3 + tx
            OW[:, t, :48] = ow2[:, :, ty, tx] @ w1
            OW[:, t, 48] = ow2[:, :, ty, tx] @ b1
    A = np.zeros((288, 9, 49), np.float32)
    for k in range(9):
        A[:, k, :48] = w2f[:, k:k + 1] * w1
        A[:, k, 48] = w2f[:, k] * b1
    Afull = np.zeros((442, 288), np.float32)
    Afull[:441] = A.reshape(288, 441).T
    Afull[441] = inputs['b2'].astype(np.float32)
    a4 = np.zeros((128, 4, 288), np.float32)
    for c in range(4):
        a4[:, c, :] = Afull[CB[c]:CB[c] + 128]
    a4[:70, 3, :] = 0.0  # chunk-3 rows 314..383 already counted in chunk 2
    # block-diag pair DFT matrices
    TfRe = np.zeros((40, 64), np.float32)
    TfIm = np.zeros((40, 64), np.float32)
    basis = np.zeros((8, 8))
    for y in range(8):
        for x in range(8):
            basis[:] = 0.0
            basis[y, x] = 1.0
            Fz = np.fft.rfft2(basis)
            TfRe[:, y * 8 + x] = Fz.real.reshape(-1)
            TfIm[:, y * 8 + x] = Fz.imag.reshape(-1)
    TiR = np.zeros((64, 40), np.float32)
    TiI = np.zeros((64, 40), np.float32)
    for j in range(40):
        fy, fx = divmod(j, 5)
        Z = np.zeros((8, 5), np.complex64)
        Z[fy, fx] = 1.0
        TiR[:, j] = np.fft.irfft2(Z, s=(8, 8)).reshape(-1)
        Z[fy, fx] = 1j
        TiI[:, j] = np.fft.irfft2(Z, s=(8, 8)).reshape(-1)
    tf2Re = np.zeros((128, 80), np.float32)
    tf2Im = np.zeros((128, 80), np.float32)
    ti2Re = np.zeros((80, 128), np.float32)
    ti2Im = np.zeros((80, 128), np.float32)
    for h in range(2):
        tf2Re[64 * h:64 * h + 64, 40 * h:40 * h + 40] = TfRe.T
        tf2Im[64 * h:64 * h + 64, 40 * h:40 * h + 40] = TfIm.T
        ti2Re[40 * h:40 * h + 40, 64 * h:64 * h + 64] = TiR.T
        ti2Im[40 * h:40 * h + 40, 64 * h:64 * h + 64] = TiI.T
    # shift matrices: S7[xin, di, xout] = 1 iff xout == xin - (di-3)
    s7 = np.zeros((128, 7, 128), np.float32)
    for di in range(7):
        d = di - 3
        for xin in range(128):
            xo = xin - d
            if 0 <= xo < 128:
                s7[xin, di, xo] = 1.0
    s3m = np.ascontiguousarray(s7[:, 2:5, :])
    tau79 = np.zeros((7, 9), np.float32)
    for di in range(7):
        for k in range(9):
            t = (di - 3) - (k % 3) + 1
            tau79[di, k] = t if abs(t) <= 2 else 99.0
    owt = np.ascontiguousarray(OW.transpose(2, 1, 0))  # [49, 9, 18]

    def b16(a):
        return np.ascontiguousarray(a).astype(_bf16)

    return dict(
        fold_ln=fold_ln,
        a4=b16(a4), owt=b16(owt),
        tf2Re=b16(tf2Re), tf2Im=b16(tf2Im),
        ti2Re=b16(ti2Re), ti2Im=b16(ti2Im),
        identb=b16(np.eye(128, dtype=np.float32)),
        s7=b16(s7), s3m=b16(s3m),
        ow1T=b16(inputs['off_w1'][:, :, 0, 0].T.astype(np.float32)),
        tau3=b16(np.tile(np.array([-1, 0, 1], np.float32), (128, 1))),
        tau5=b16(np.tile(np.arange(-2, 3, dtype=np.float32), (128, 1))),
        tau79y=b16(np.tile(tau79.reshape(1, 7, 9, 1), (128, 1, 1, BND))),
        lnw=b16(np.tile(inputs['ln_w'].astype(np.float32), (128, 1))),
        lnb=b16(np.tile(inputs['ln_b'].astype(np.float32), (128, 1))),
        wo50=b16(np.concatenate(
            [(inputs['w3'][:, :, 0, 0] * lnscale[None, :]).T,
             (inputs['off_w3'][:, :, 0, 0] * lnscale[None, :]).T],
            axis=1).astype(np.float32)),
        b3r=np.tile(inputs['b3'].astype(np.float32)[None, :], (128, 1)),
    )


def make_in_maps(inputs):
    C = _hat_consts(inputs)
    x = np.asarray(inputs['x'][0], np.float32)
    xp = np.zeros((48, H + 8, W), np.float32)
    xp[:, 4:4 + H, :] = x
    xpb = xp.astype(_bf16)
    in_a = []
    for ci in range(NCORES):
        onesp = np.zeros((128, NYS), np.float32)
        for j in range(NYS):
            if 0 <= 16 * ci - 3 + j < H:
                onesp[:, j] = 1.0
        in_a.append(dict(
            xh=np.ascontiguousarray(xpb[:, 16 * ci + 1:16 * ci + 1 + NYS, :]),
            xt=np.ascontiguousarray(
                xpb[:, 16 * ci:16 * ci + NYX, :].transpose(2, 0, 1)),
            onesp=onesp.astype(_bf16),
            identb=C['identb'], ow1T=C['ow1T'], owt=C['owt'], a4=C['a4'],
            tf2Re=C['tf2Re'], tf2Im=C['tf2Im'], ti2Re=C['ti2Re'],
            ti2Im=C['ti2Im'], s7=C['s7'], tau3=C['tau3'], tau5=C['tau5'],
            tau79y=C['tau79y'], lnw=C['lnw'], lnb=C['lnb']))
    return C, in_a


def unscramble_vo(res_a):
    """[128, 16, 96] pixel-major per core -> [96, H, 128] channel-major."""
    parts = []
    for r in res_a:
        vp = r['vo_out'].reshape(2, 8, 8, 16, 96)   # [ph, py, px, pc, c]
        parts.append(vp.transpose(4, 0, 1, 3, 2).reshape(96, BND, 128))
    return np.concatenate(parts, axis=1)


def make_in_maps_b(C, vo_full):
    vop = np.zeros((96, H + 2, 128), _bf16)
    vop[:, 1:1 + H, :] = vo_full
    in_b = []
    for ci in range(NCORES):
        in_b.append(dict(
            voh=np.ascontiguousarray(vop[:, 16 * ci:16 * ci + NYV, :]),
            wo50=C['wo50'], b3r=C['b3r'], tau3=C['tau3'], s3m=C['s3m']))
    return in_b


_CACHE = {}


def kernel(**inputs):
    C, in_a = make_in_maps(inputs)
    key = 'nca_fold' if C['fold_ln'] else 'nca'
    if key not in _CACHE:
        _CACHE[key] = build_kernel_a(fold_ln=C['fold_ln'])
        _CACHE.setdefault('ncb', build_kernel_b())
    nca, ncb = _CACHE[key], _CACHE['ncb']
    res_a = bass_utils.run_bass_kernel_spmd(nca, in_a, core_ids=list(range(NCORES)))
    vo_full = unscramble_vo(res_a.results)
    in_b = make_in_maps_b(C, vo_full)
    res_b = bass_utils.run_bass_kernel_spmd(ncb, in_b, core_ids=list(range(NCORES)))
    # outp [128, 48, 16] x-major -> [48, 16, 128]
    out = np.concatenate([r['outp'].transpose(1, 2, 0) for r in res_b.results],
                         axis=1)
    return out[None].astype(np.float32)


# revision 57
# speedup vs baseline: 1.8065x; 1.8065x over previous
"""Trainium2 Bass kernel for nn_DeformAttn (deformable attention, patch-FFT).

Self-contained: hardcodes shapes for x [1,48,128,128], 8 NeuronCores,
y-band split (16 rows/core).

Design (per core):
- All deformable convs use dense hat-tap sampling with COLUMN-SHIFTED weight
  evaluation: for each x-shift delta, the 2D hat weights (hy*hx, both read
  from the delta-shifted offset field so they are evaluated at the OUTPUT
  pixel) premultiply the unshifted source on DVE (a slice of the work goes
  to the GpSimd/Pool engine); the x-shift-and-add runs on the PE as banded
  0/1 shift matmuls accumulating in PSUM, with the y-tap reduction riding
  the same PSUM accumulation (one matmul per ty plane).
- Layer-1 (1x1 deform, |off|<1): 3x3 taps.  Layer-2 (3x3 depthwise deform,
  |off|<2): 5x5 taps per kernel point, 7 global x-shifts.  Layer-3: 3x3 taps.
- qkv = A^T u with host-precomputed Khatri-Rao A (w2 (x) W1), bias via
  indicator/ones rows.  off2 via OW matrix on channel-major s1x.
- Hat weights are built on the UNSHIFTED offset field (one abs/relu pass)
  and then column-shifted by SBUF->SBUF DMAs (hy) / PE shift-matmuls
  (off1/off3); per-delta hat products chase the shift DMAs.
- x arrives in BOTH channel-major and x-major layouts from the host (no
  on-device input transposes).  vo leaves pixel-major; the host transposes.
- Patch FFT (8x8 circular conv): patch-PAIR block-diagonal real-DFT matmuls
  (128 partitions = 2 patches), complex pointwise on DVE, block-diag inverse.
- Everything bf16 on-chip except PSUM accumulation and LN statistics.
"""
import numpy as np
import ml_dtypes
_bf16 = np.float16
from contextlib import ExitStack

import concourse.bacc as bacc
import concourse.mybir as mybir
import concourse.tile as tile
from concourse.bass import AP
from concourse import bass_utils

dt = mybir.dt
F32 = dt.float32
BF16 = dt.float16
ALU = mybir.AluOpType
ACTF = mybir.ActivationFunctionType
AX = mybir.AxisListType

H = W = 128
BND = 16          # band rows per core
NYS = 22          # s1x rows per core (band +/-3)
NYX = 24          # x rows per core (band +/-4)
NYV = 18          # vo rows per core (band +/-1)
NCORES = 8
CB = [0, 128, 256, 314]   # uT K-chunk bases over 442 slots

def _v(t, off, dims):
    """View of tile t: keep its full partition dim, custom free dims."""
    return AP(t.tensor, t.offset + off, [list(t.ap[0])] + [list(d) for d in dims])


def _vp(t, p0, np_, off, dims):
    """View with partition sub-range [p0, p0+np_) and custom free dims."""
    st = t.ap[0][0]
    return AP(t.tensor, t.offset + p0 * st + off,
              [[st, np_]] + [list(d) for d in dims])


def build_kernel_a(fold_ln=False):
    nc = bacc.Bacc("TRN2", target_bir_lowering=False, debug=False)
    xh_d = nc.dram_tensor("xh", [48, NYS, W], BF16, kind="ExternalInput")
    xt_d = nc.dram_tensor("xt", [128, 48, NYX], BF16, kind="ExternalInput")
    onesp_d = nc.dram_tensor("onesp", [128, NYS], BF16, kind="ExternalInput")
    identb_d = nc.dram_tensor("identb", [128, 128], BF16, kind="ExternalInput")
    ow1T_d = nc.dram_tensor("ow1T", [48, 2], BF16, kind="ExternalInput")
    owt_d = nc.dram_tensor("owt", [49, 9, 18], BF16, kind="ExternalInput")
    a4_d = nc.dram_tensor("a4", [128, 4, 288], BF16, kind="ExternalInput")
    tf2Re_d = nc.dram_tensor("tf2Re", [128, 80], BF16, kind="ExternalInput")
    tf2Im_d = nc.dram_tensor("tf2Im", [128, 80], BF16, kind="ExternalInput")
    ti2Re_d = nc.dram_tensor("ti2Re", [80, 128], BF16, kind="ExternalInput")
    ti2Im_d = nc.dram_tensor("ti2Im", [80, 128], BF16, kind="ExternalInput")
    s7_d = nc.dram_tensor("s7", [128, 7, 128], BF16, kind="ExternalInput")
    tau3_d = nc.dram_tensor("tau3", [128, 3], BF16, kind="ExternalInput")
    tau5_d = nc.dram_tensor("tau5", [128, 5], BF16, kind="ExternalInput")
    tau79y_d = nc.dram_tensor("tau79y", [128, 7, 9, BND], BF16,
                              kind="ExternalInput")
    lnw_d = nc.dram_tensor("lnw", [128, 96], BF16, kind="ExternalInput")
    lnb_d = nc.dram_tensor("lnb", [128, 96], BF16, kind="ExternalInput")
    vo_out = nc.dram_tensor("vo_out", [128, BND, 96], BF16,
                            kind="ExternalOutput")

    with tile.TileContext(nc) as tc, ExitStack() as top:
        cpool = top.enter_context(tc.tile_pool(name="consts", bufs=1))
        xh = cpool.tile([48, NYS, W], BF16)
        xt = cpool.tile([128, 48, NYX], BF16)
        identb = cpool.tile([128, 128], BF16)
        ow1T = cpool.tile([48, 2], BF16)
        owt = cpool.tile([49, 9, 18], BF16)
        a4 = cpool.tile([128, 4, 288], BF16)
        tf2Re = cpool.tile([128, 80], BF16)
        tf2Im = cpool.tile([128, 80], BF16)
        ti2Re = cpool.tile([80, 128], BF16)
        ti2Im = cpool.tile([80, 128], BF16)
        s7 = cpool.tile([128, 7, 128], BF16)
        tau3 = cpool.tile([128, 3], BF16)
        tau5 = cpool.tile([128, 5], BF16)
        tau79y = cpool.tile([128, 7, 9, BND], BF16)
        lnw = cpool.tile([128, 96], BF16)
        lnb = cpool.tile([128, 96], BF16)
        onesp = cpool.tile([128, NYS], BF16, name="onesp")
        # one queue (the model serializes HWDGE anyway; issuing from Act
        # stalls the Act sequencer), ordered by first use
        first = [(xh, xh_d), (ow1T, ow1T_d), (identb, identb_d),
                 (s7, s7_d), (xt, xt_d),
                 (tau3, tau3_d), (onesp, onesp_d),
                 (owt, owt_d), (tau5, tau5_d), (tau79y, tau79y_d),
                 (a4, a4_d), (tf2Re, tf2Re_d), (tf2Im, tf2Im_d),
                 (ti2Re, ti2Re_d), (ti2Im, ti2Im_d)]
        if not fold_ln:
            first += [(lnw, lnw_d), (lnb, lnb_d)]
        for sb, dr in first:
            nc.sync.dma_start(sb[:], dr[:])

        scr = cpool.tile([1, 8], F32, name="scr")
        nc.vector.memset(scr[:], 1.0)
        nc.scalar.activation(scr[:], scr[:], ACTF.Sqrt)

        psum = top.enter_context(tc.tile_pool(name="psum", bufs=3, space="PSUM"))
        pupool = top.enter_context(tc.tile_pool(name="pu", bufs=4, space="PSUM"))

        big_cm = tc.tile_pool(name="big", bufs=1)
        bpool = big_cm.__enter__()
        s1x = bpool.tile([128, 49, NYS], BF16)
        u = bpool.tile([128, 442, BND], BF16)
        uT = bpool.tile([128, 4, 16, 128], BF16)
        qkv = bpool.tile([128, 16, 288], BF16)

        # ================= phase X: off1, layer-1 ==========================
        with tc.tile_pool(name="px", bufs=1) as p1:
            # off1 on s1x rows -> one psum, region-accumulated
            po = pupool.tile([128, 512], F32, tag="acc", name="po", bufs=3)
            for y in range(NYS):
                nc.tensor.matmul(po[:128, 2 * y:2 * y + 2],
                                 _v(xh, y * W, [[1, 128]]),
                                 ow1T[:], start=True, stop=True)
            off1pm = p1.tile([128, NYS, 2], BF16)
            nc.scalar.copy(off1pm[:], po[:128, :44])
            # shifted offset copies via PE shift-matmuls (edges auto-zero):
            # slot d=-1 (di=0): w[xin] = off[xin+1]; slot d=+1: off[xin-1]
            off1s3 = p1.tile([128, 3, NYS, 2], BF16)
            nc.vector.tensor_copy(off1s3[:, 1], off1pm[:])
            po1s = pupool.tile([128, 512], F32, tag="acc", name="po1s", bufs=3)
            nc.tensor.matmul(po1s[:128, 0:44], _v(s7, 4 * 128, [[1, 128]]),
                             _v(off1pm, 0, [[1, 44]]), start=True, stop=True)
            nc.tensor.matmul(po1s[:128, 44:88], _v(s7, 2 * 128, [[1, 128]]),
                             _v(off1pm, 0, [[1, 44]]), start=True, stop=True)
            nc.scalar.copy(_v(off1s3, 0, [[2 * 2 * NYS, 2], [1, 44]]),
                           po1s[:128, :88])
            # warm the PE p-state while W1d/tmp1 are being built so the
            # layer-1 matmuls run at full clock (junk transposes, never read)
            jp = psum.tile([128, 1024], BF16, tag="psb", name="jp", bufs=3)
            for _ in range(38):
                nc.tensor.transpose(jp[:128, :128], identb[:, :], identb[:, :])
            # W1d [128, 3d, 3ty, 22] = hat(oy_sh - (ty-1)) * hat(ox_sh - d)
            w1d = p1.tile([128, 3, 3, NYS], BF16)
            hx1 = p1.tile([128, 3, NYS], BF16)
            with nc.allow_low_precision(reason="hat weights bf16"):
                nc.vector.tensor_tensor(
                    out=w1d[:],
                    in0=_v(off1s3, 0, [[2 * NYS, 3], [0, 3], [2, NYS]]),
                    in1=_v(tau3, 0, [[0, 3], [1, 3], [0, NYS]]),
                    op=ALU.subtract)
                nc.scalar.activation(w1d[:], w1d[:], ACTF.Abs)
                nc.scalar.activation(w1d[:], w1d[:], ACTF.Relu,
                                     bias=1.0, scale=-1.0)
                nc.vector.tensor_tensor(
                    out=hx1[:],
                    in0=_v(off1s3, 1, [[2 * NYS, 3], [2, NYS]]),
                    in1=_v(tau3, 0, [[1, 3], [0, NYS]]),
                    op=ALU.subtract)
                nc.scalar.activation(hx1[:], hx1[:], ACTF.Abs)
                nc.scalar.activation(hx1[:], hx1[:], ACTF.Relu,
                                     bias=1.0, scale=-1.0)
                nc.vector.tensor_tensor(
                    out=w1d[:], in0=w1d[:],
                    in1=_v(hx1, 0, [[NYS, 3], [0, 3], [1, NYS]]),
                    op=ALU.mult)
            # layer-1 sampling: premult per delta, PE shift-accumulate
            ps1 = [pupool.tile([128, 512], F32, tag="acc", bufs=3, name=f"ps1_{c}")
                   for c in range(3)]
            tmp1s = []
            for di in range(3):
                tmp1 = p1.tile([128, 3, 48, NYS], BF16, tag="tmp1",
                               name="tmp1", bufs=3)
                with nc.allow_low_precision(reason="sampling taps bf16"):
                    nc.vector.tensor_tensor(
                        out=tmp1[:],
                        in0=_v(xt, 0, [[1, 3], [NYX, 48], [1, NYS]]),
                        in1=_v(w1d, di * 3 * NYS, [[NYS, 3], [0, 48], [1, NYS]]),
                        op=ALU.mult)
                tmp1s.append(tmp1)
            for di in range(3):
                tmp1 = tmp1s[di]
                lhs = _v(s7, (di + 2) * 128, [[1, 128]])  # delta=-1,0,1 -> slots 2,3,4
                for ty in range(3):
                    for ch in range(3):
                        nc.tensor.matmul(
                            ps1[ch][:128, :352], lhs,
                            _v(tmp1, ty * 48 * NYS + ch * 352, [[1, 352]]),
                            start=(di == 0 and ty == 0),
                            stop=(di == 2 and ty == 2))
            for ch in range(3):
                (nc.scalar.copy if ch != 1 else nc.vector.tensor_copy)(
                    _v(s1x, ch * 352, [[1, 352]]), ps1[ch][:128, :352])
            nc.sync.dma_start(_v(s1x, 48 * NYS, [[1, NYS]]), onesp[:])

        # ================= phase O: s1xT, off2, W2d =========================
        w2ds = [bpool.tile([128, 9, 5, BND], BF16, name=f"w2d{di}")
                for di in range(7)]
        with tc.tile_pool(name="po2", bufs=1) as p2:
            hox7 = p2.tile([128, 7, 864], BF16)
            nc.gpsimd.memset(hox7[:], 0.0)
            s1xT = p2.tile([49, NYS, 130], BF16)
            nc.gpsimd.memset(_v(s1xT, 0, [[130, NYS], [1, 1]]), 0.0)
            nc.gpsimd.memset(_v(s1xT, 129, [[130, NYS], [1, 1]]), 0.0)
            for gi, (g0, gn) in enumerate([(0, 8), (8, 8), (16, 6)]):
                ps = psum.tile([128, 1024], BF16, tag="psb", name="ps", bufs=3)
                for i in range(gn):
                    nc.tensor.transpose(ps[:49, i * 128:(i + 1) * 128],
                                        _v(s1x, g0 + i, [[NYS, 49]]),
                                        identb[:, :])
                dst = _v(s1xT, g0 * 130 + 1, [[130, gn], [1, 128]])
                (nc.scalar.copy if gi % 2 == 0 else nc.vector.tensor_copy)(
                    dst, ps[:49, :gn * 128])
            # off2: per band row b, 9 taps accumulate; 2 psum region-tiles
            pofs = [pupool.tile([128, 512], F32, tag="acc", bufs=3, name=f"po2_{h}")
                    for h in range(2)]
            for b in range(BND):
                po2 = pofs[b // 8]
                col = 18 * (b % 8)
                for t in range(9):
                    ty, tx = divmod(t, 3)
                    nc.tensor.matmul(
                        po2[:128, col:col + 18],
                        _v(s1xT, (b + 2 + ty) * 130 + tx, [[1, 128]]),
                        owt[:, t], start=(t == 0), stop=(t == 8))
            off2pm = p2.tile([128, BND, 18], BF16)
            nc.scalar.copy(_v(off2pm, 0, [[1, 144]]), pofs[0][:128, :144])
            nc.scalar.copy(_v(off2pm, 144, [[1, 144]]), pofs[1][:128, :144])
            # base hat_y + compact ox on the UNSHIFTED field, side by side:
            # hob = [hyb (720) | oxb (144)]
            hob = p2.tile([128, 864], BF16)
            with nc.allow_low_precision(reason="hat weights bf16"):
                nc.vector.tensor_tensor(
                    out=_v(hob, 0, [[80, 9], [16, 5], [1, BND]]),
                    in0=_v(off2pm, 0, [[2, 9], [0, 5], [18, BND]]),
                    in1=_v(tau5, 0, [[0, 9], [1, 5], [0, BND]]),
                    op=ALU.subtract)
                nc.vector.tensor_copy(
                    _v(hob, 720, [[16, 9], [1, BND]]),
                    _v(off2pm, 1, [[2, 9], [18, BND]]))
                nc.scalar.activation(_v(hob, 0, [[1, 720]]),
                                     _v(hob, 0, [[1, 720]]), ACTF.Abs)
                nc.scalar.activation(_v(hob, 0, [[1, 720]]),
                                     _v(hob, 0, [[1, 720]]), ACTF.Relu,
                                     bias=1.0, scale=-1.0)
            # column-shifted copies hox7[xin, di] = hob[xin - d] via DMAs;
            # edge partitions stay zero from the early Pool memset.  hxb and
            # the w2d slice for each di chase its shift DMA.
            hxb = p2.tile([128, 7, 9, BND], BF16)
            for d in range(-3, 4):
                di = d + 3
                if d > 0:
                    nc.sync.dma_start(hox7[d:128, di], hob[0:128 - d])
                elif d < 0:
                    nc.sync.dma_start(hox7[0:128 + d, di], hob[-d:128])
                else:
                    nc.sync.dma_start(hox7[:, di], hob[:])
                hxd = _v(hxb, di * 9 * BND, [[BND, 9], [1, BND]])
                with nc.allow_low_precision(reason="hat weights bf16"):
                    nc.gpsimd.tensor_tensor(
                        out=hxd,
                        in0=_v(hox7, di * 864 + 720, [[16, 9], [1, BND]]),
                        in1=_v(tau79y, di * 9 * BND, [[BND, 9], [1, BND]]),
                        op=ALU.subtract)
                    nc.scalar.activation(hxd, hxd, ACTF.Abs)
                    nc.scalar.activation(hxd, hxd, ACTF.Relu,
                                         bias=1.0, scale=-1.0)
                    nc.vector.tensor_tensor(
                        out=w2ds[di][:],
                        in0=_v(hox7, di * 864, [[80, 9], [16, 5], [1, BND]]),
                        in1=_v(hxb, di * 9 * BND, [[BND, 9], [0, 5], [1, BND]]),
                        op=ALU.mult)

        # keep the PE p-state warm across the hox7 shift-DMA stall so the
        # first sampling groups run at full clock
        jp2 = psum.tile([128, 1024], BF16, tag="psb", name="jp2", bufs=3)
        for _ in range(48):
            nc.tensor.transpose(jp2[:128, :128], identb[:, :], identb[:, :])

        # ========== phase S: sampling (pipelined premults, uT dribbled) ====
        # Premults for group k+1 (DVE) / k+2 (Pool) issue while the PE
        # consumes group k's 50 stall-free matmuls; uT transposes fill the
        # inter-group gaps so the PE p-state stays ramped.
        DVE_S = [0, 1, 3, 4]
        MM_S = [0, 1, 3, 4, 2]

        def emit_uT_group(c, g, eng):
            # c 0..2: full 128-slot chunks; c 3: slots 378..441 (64; slots
            # 378..383 are zeroed in a4 since chunk 2 already counts them)
            base, nsl = (CB[c], 128) if c < 3 else (378, 64)
            ps = psum.tile([128, 1024], BF16, tag="psb", name="ps", bufs=3)
            for yy in range(8):
                y = g * 8 + yy
                nc.tensor.transpose(
                    ps[:nsl, yy * 128:(yy + 1) * 128],
                    _v(u, base * BND + y, [[BND, nsl]]),
                    identb[:, :])
            dst = _vp(uT, 0, nsl, c * 2048 + g * 64,
                      [[8, 8], [128, 16], [1, 8]])
            (nc.scalar.copy if eng == 0 else nc.vector.tensor_copy)(
                dst, ps[:nsl, :1024])

        with tc.tile_pool(name="psmp", bufs=1) as p3:
            tmps = {}

            def emit_pool_premult(k):
                ki, kj = divmod(k, 3)
                tmp = p3.tile([128, 5, 49, BND], BF16, tag="tmpp",
                              name="tmpp", bufs=3)
                with nc.allow_low_precision(reason="sampling taps bf16"):
                    nc.gpsimd.tensor_tensor(
                        out=tmp[:],
                        in0=_v(s1x, ki, [[1, 5], [NYS, 49], [1, BND]]),
                        in1=_v(w2ds[kj + 2], k * 5 * BND,
                               [[BND, 5], [0, 49], [1, BND]]),
                        op=ALU.mult)
                tmps[(k, 2)] = tmp

            def emit_dve_premults(k):
                ki, kj = divmod(k, 3)
                for s in DVE_S:
                    tmp = p3.tile([128, 5, 49, BND], BF16, tag="tmp",
                                  name="tmp", bufs=10)
                    with nc.allow_low_precision(reason="sampling taps bf16"):
                        nc.vector.tensor_tensor(
                            out=tmp[:],
                            in0=_v(s1x, ki, [[1, 5], [NYS, 49], [1, BND]]),
                            in1=_v(w2ds[kj + s], k * 5 * BND,
                                   [[BND, 5], [0, 49], [1, BND]]),
                            op=ALU.mult)
                    tmps[(k, s)] = tmp

            # (chunk, group) transposes dribbled after group k's matmuls
            UT_SCHED = {3: [(0, 0)], 4: [(0, 1)], 6: [(1, 0)], 7: [(1, 1)],
                        8: [(2, 0), (2, 1)]}

            emit_pool_premult(0)
            emit_pool_premult(1)
            emit_dve_premults(0)
            for k in range(9):
                ki, kj = divmod(k, 3)
                if k < 8:
                    emit_dve_premults(k + 1)
                if k < 7:
                    emit_pool_premult(k + 2)
                if k < 8:
                    for c, g in UT_SCHED.get(k, []):
                        emit_uT_group(c, g, 0)
                pk = [pupool.tile([128, 512], F32, tag="acc", bufs=3,
                                  name=f"pk{ch}") for ch in range(2)]
                mm_order = [0, 1, 2, 3, 4] if k == 8 else MM_S
                for si, s in enumerate(mm_order):
                    di = kj + s
                    tmp = tmps.pop((k, s))
                    lhs = _v(s7, di * 128, [[1, 128]])
                    for ty in range(5):
                        for ch in range(2):
                            nc.tensor.matmul(
                                pk[ch][:128, :392], lhs,
                                _v(tmp, ty * 784 + ch * 392, [[1, 392]]),
                                start=(si == 0 and ty == 0),
                                stop=(si == 4 and ty == 4))
                nc.scalar.copy(_v(u, k * 784, [[1, 392]]),
                               pk[0][:128, :392])
                (nc.vector.tensor_copy if k == 8 else nc.scalar.copy)(
                    _v(u, k * 784 + 392, [[1, 392]]), pk[1][:128, :392])
                if k == 8:
                    nc.vector.memset(_v(u, 441 * BND, [[1, BND]]), 1.0)
                    for ci, (c, g) in enumerate(UT_SCHED[8]):
                        emit_uT_group(c, g, ci % 2)

        # ================= phase Q + FFT ====================================
        with tc.tile_pool(name="pfft", bufs=1) as fp:
            qhRe = fp.tile([80, 16, 192], BF16)
            qhIm = fp.tile([80, 16, 192], BF16)
            for g in range(2):
                emit_uT_group(3, g, g % 2)
            for pc in range(16):
                qp = pupool.tile([128, 512], F32, tag="acc", bufs=3,
                                 name="qp")
                for c in range(3):
                    nc.tensor.matmul(qp[:128, :288],
                                     _v(uT, c * 2048 + pc * 128, [[1, 128]]),
                                     a4[:, c], start=(c == 0), stop=False)
                nc.tensor.matmul(qp[:128, :288],
                                 _vp(uT, 0, 64, 3 * 2048 + pc * 128,
                                     [[1, 128]]),
                                 _vp(a4, 0, 64, 3 * 288, [[1, 288]]),
                                 start=False, stop=True)
                if pc % 2 == 0:
                    nc.scalar.copy(_v(qkv, pc * 288, [[1, 288]]),
                                   qp[:128, :288])
                else:
                    nc.vector.tensor_copy(_v(qkv, pc * 288, [[1, 288]]),
                                          qp[:128, :288])
                if pc % 2 == 1:
                    g = pc // 2
                    rhs = _v(qkv, 2 * g * 288, [[288, 2], [1, 192]])
                    psR = psum.tile([128, 512], F32, tag="ps", name="ps",
                                    bufs=2)
                    nc.tensor.matmul(psR[:80, :384], tf2Re[:], rhs,
                                     start=True, stop=True)
                    nc.scalar.copy(_v(qhRe, 2 * g * 192, [[1, 384]]),
                                   psR[:80, :384])
                    psI = psum.tile([128, 512], F32, tag="ps", name="ps",
                                    bufs=2)
                    nc.tensor.matmul(psI[:80, :384], tf2Im[:], rhs,
                                     start=True, stop=True)
                    (nc.vector.tensor_copy if g % 2 == 0 else nc.scalar.copy)(
                        _v(qhIm, 2 * g * 192, [[1, 384]]), psI[:80, :384])
            # complex pointwise + inverse DFT + LN partial stats, pipelined
            # per quarter (4 patch-pairs each)
            ohRe = fp.tile([80, 16, 96], BF16)
            ohIm = fp.tile([80, 16, 96], BF16)
            t1 = fp.tile([80, 16, 96], BF16)
            t2 = fp.tile([80, 16, 96], BF16)
            t1b = fp.tile([80, 16, 96], BF16)
            t2b = fp.tile([80, 16, 96], BF16)
            osb = fp.tile([128, 16, 96], BF16)
            ssum = fp.tile([128, BND], F32)
            sqs = fp.tile([128, BND], F32)
            sq = fp.tile([128, 16, 96], BF16)
            with nc.allow_low_precision(reason="fft products bf16"):
                for q in range(4):
                    o = q * 4 * 192
                    arq = _v(qhRe, o, [[192, 4], [1, 96]])
                    brq = _v(qhRe, o + 96, [[192, 4], [1, 96]])
                    aiq = _v(qhIm, o, [[192, 4], [1, 96]])
                    biq = _v(qhIm, o + 96, [[192, 4], [1, 96]])
                    oq = q * 384
                    t1q = _v(t1, oq, [[1, 384]])
                    t2q = _v(t2, oq, [[1, 384]])
                    t1bq = _v(t1b, oq, [[1, 384]])
                    t2bq = _v(t2b, oq, [[1, 384]])
                    nc.vector.tensor_tensor(out=t1q, in0=arq, in1=brq,
                                            op=ALU.mult)
                    nc.gpsimd.tensor_tensor(out=t2q, in0=aiq, in1=biq,
                                            op=ALU.mult)
                    nc.vector.tensor_tensor(out=_v(ohRe, oq, [[1, 384]]),
                                            in0=t1q, in1=t2q, op=ALU.subtract)
                    nc.gpsimd.tensor_tensor(out=t2bq, in0=aiq, in1=brq,
                                            op=ALU.mult)
                    nc.vector.tensor_tensor(out=t1bq, in0=arq, in1=biq,
                                            op=ALU.mult)
                    nc.vector.tensor_tensor(out=_v(ohIm, oq, [[1, 384]]),
                                            in0=t1bq, in1=t2bq, op=ALU.add)
                    ps = psum.tile([128, 512], F32, tag="ps", name="ps", bufs=2)
                    nc.tensor.matmul(ps[:128, :384], ti2Re[:],
                                     _v(ohRe, oq, [[1, 384]]),
                                     start=True, stop=False)
                    nc.tensor.matmul(ps[:128, :384], ti2Im[:],
                                     _v(ohIm, oq, [[1, 384]]),
                                     start=False, stop=True)
                    (nc.scalar.copy if q % 2 == 0 else nc.vector.tensor_copy)(
                        _v(osb, oq, [[1, 384]]), ps[:128, :384])
                    nc.vector.tensor_reduce(
                        out=_v(ssum, q * 4, [[1, 4]]),
                        in_=_v(osb, oq, [[96, 4], [1, 96]]),
                        axis=AX.X, op=ALU.add)
                    nc.scalar.activation(_v(sq, oq, [[1, 384]]),
                                         _v(osb, oq, [[1, 384]]), ACTF.Square)
                    nc.vector.tensor_reduce(
                        out=_v(sqs, q * 4, [[1, 4]]),
                        in_=_v(sq, oq, [[96, 4], [1, 96]]),
                        axis=AX.X, op=ALU.add)
            # LayerNorm stats -> normalize -> gate by v, half at a time
            mu = fp.tile([128, BND], F32)
            var = fp.tile([128, BND], F32)
            musq = fp.tile([128, BND], F32)
            mub = fp.tile([128, BND, 96], BF16)
            invb = fp.tile([128, BND, 96], BF16)
            on = fp.tile([128, 16, 96], BF16)
            vo = fp.tile([128, 16, 96], BF16)
            nc.scalar.activation(mu[:], ssum[:], ACTF.Copy, scale=1.0 / 96)
            nc.scalar.activation(var[:], sqs[:], ACTF.Copy, scale=1.0 / 96)
            nc.scalar.activation(musq[:], mu[:], ACTF.Square)
            nc.vector.tensor_tensor(out=var[:], in0=var[:], in1=musq[:],
                                    op=ALU.subtract)
            std = fp.tile([128, BND], F32)
            inv = fp.tile([128, BND], F32)
            nc.vector.tensor_scalar_add(out=var[:], in0=var[:], scalar1=1e-5)
            nc.scalar.activation(std[:], var[:], ACTF.Sqrt)
            nc.vector.reciprocal(inv[:], std[:])
            nc.scalar.copy(mub[:], _v(mu, 0, [[1, BND], [0, 96]]))
            nc.scalar.copy(invb[:], _v(inv, 0, [[1, BND], [0, 96]]))
            with nc.allow_low_precision(reason="ln bf16"):
                for h in range(2):
                    oh = h * 8 * 96
                    onh = _v(on, oh, [[1, 768]])
                    nc.vector.tensor_tensor(
                        out=onh, in0=_v(osb, oh, [[1, 768]]),
                        in1=_v(mub, oh, [[1, 768]]),
                        op=ALU.subtract)
                    nc.vector.tensor_tensor(
                        out=onh, in0=onh,
                        in1=_v(invb, oh, [[1, 768]]),
                        op=ALU.mult)
                    if not fold_ln:
                        nc.vector.tensor_tensor(
                            out=onh, in0=onh,
                            in1=_v(lnw, 0, [[0, 8], [1, 96]]), op=ALU.mult)
                        nc.vector.tensor_tensor(
                            out=onh, in0=onh,
                            in1=_v(lnb, 0, [[0, 8], [1, 96]]), op=ALU.add)
                    nc.vector.tensor_tensor(
                        out=_v(vo, oh, [[1, 768]]), in0=onh,
                        in1=_v(qkv, 192 + h * 8 * 288, [[288, 8], [1, 96]]),
                        op=ALU.mult)
                    nc.sync.dma_start(vo_out[:, 8 * h:8 * (h + 1)],
                                      _v(vo, oh, [[1, 768]]))
        big_cm.__exit__(None, None, None)
    nc.compile()
    return nc


def build_kernel_b():
    """Layer-3: out = W3 @ deform_sample(vo) + b3 == deform_sample(W3 @ vo) + b3
    (sampling offsets are channel-shared, so the contraction commutes).
    W3-contraction and off3 ride ONE fused per-row matmul; output leaves
    x-major (host transposes back)."""
    nc = bacc.Bacc("TRN2", target_bir_lowering=False, debug=False)
    voh_d = nc.dram_tensor("voh", [96, NYV, 128], BF16, kind="ExternalInput")
    wo50_d = nc.dram_tensor("wo50", [96, 50], BF16, kind="ExternalInput")
    b3r_d = nc.dram_tensor("b3r", [128, 48], F32, kind="ExternalInput")
    tau3_d = nc.dram_tensor("tau3", [128, 3], BF16, kind="ExternalInput")
    s3_d = nc.dram_tensor("s3m", [128, 3, 128], BF16, kind="ExternalInput")
    out_d = nc.dram_tensor("outp", [128, 48, BND], F32, kind="ExternalOutput")
    with tile.TileContext(nc) as tc, ExitStack() as top:
        pool = top.enter_context(tc.tile_pool(name="b", bufs=1))
        pup = top.enter_context(tc.tile_pool(name="pub", bufs=4, space="PSUM"))
        voh = pool.tile([96, NYV, 128], BF16)
        wo50 = pool.tile([96, 50], BF16)
        b3r = pool.tile([128, 48], F32)
        tau3 = pool.tile([128, 3], BF16)
        s3m = pool.tile([128, 3, 128], BF16)
        nc.sync.dma_start(wo50[:], wo50_d[:])
        for rc in range(3):
            nc.sync.dma_start(voh[:, 6 * rc:6 * (rc + 1)],
                              voh_d[:, 6 * rc:6 * (rc + 1)])
        for sb, dr in [(tau3, tau3_d), (s3m, s3_d), (b3r, b3r_d)]:
            nc.sync.dma_start(sb[:], dr[:])
        # fused [W3 | ow3] per-row contraction -> wvx2 [128, 50, 18] c-major
        wvx2 = pool.tile([128, 50, NYV], BF16)
        pws = [pup.tile([128, 512], F32, tag="acc", bufs=4, name=f"pw{h}")
               for h in range(2)]
        for y in range(NYV):
            nc.tensor.matmul(pws[y // 9][:128, (y % 9) * 50:(y % 9) * 50 + 50],
                             _v(voh, y * 128, [[1, 128]]),
                             wo50[:], start=True, stop=True)
        for h in range(2):
            dst = _v(wvx2, h * 9, [[1, 9], [NYV, 50]])
            (nc.scalar.copy if h == 0 else nc.vector.tensor_copy)(
                dst, pws[h][:128, :450])
        # off3 values live at wvx2 cols 48:50: off3[x, o, y] = wvx2[x, 48+o, y]
        off3s3 = pool.tile([128, 3, BND, 2], BF16)
        # center (d=0): off3s3[:, 1, b, o] = off3[x, o, b+1]
        nc.vector.tensor_copy(
            off3s3[:, 1], _v(wvx2, 48 * NYV + 1, [[1, BND], [NYV, 2]]))
        # shifted slots via PE shift-matmuls (edges auto-zero):
        # slot 0 (d=-1): off[xin+1] -> s3m slot 2; slot 2 (d=+1): slot 0
        p3s = pup.tile([128, 512], F32, tag="acc", name="p3s", bufs=4)
        nc.tensor.matmul(p3s[:128, 0:32], _v(s3m, 2 * 128, [[1, 128]]),
                         _v(off3s3, 32, [[1, 32]]), start=True, stop=True)
        nc.tensor.matmul(p3s[:128, 32:64], _v(s3m, 0, [[1, 128]]),
                         _v(off3s3, 32, [[1, 32]]), start=True, stop=True)
        nc.scalar.copy(_v(off3s3, 0, [[64, 2], [1, 32]]), p3s[:128, :64])
        # warm the PE p-state while the hat weights are built (s3m slot 1
        # is the identity matrix; junk transposes, never read)
        jb = pup.tile([128, 1024], BF16, tag="jnk", name="jb", bufs=1)
        for _ in range(34):
            nc.tensor.transpose(jb[:128, :128], _v(s3m, 128, [[1, 128]]),
                                _v(s3m, 128, [[1, 128]]))
        w3d = pool.tile([128, 3, 3, BND], BF16)
        hx3 = pool.tile([128, 3, BND], BF16)
        with nc.allow_low_precision(reason="hat weights bf16"):
            nc.vector.tensor_tensor(
                out=w3d[:],
                in0=_v(off3s3, 0, [[2 * BND, 3], [0, 3], [2, BND]]),
                in1=_v(tau3, 0, [[0, 3], [1, 3], [0, BND]]),
                op=ALU.subtract)
            nc.scalar.activation(w3d[:], w3d[:], ACTF.Abs)
            nc.scalar.activation(w3d[:], w3d[:], ACTF.Relu, bias=1.0, scale=-1.0)
            nc.vector.tensor_tensor(
                out=hx3[:],
                in0=_v(off3s3, 1, [[2 * BND, 3], [2, BND]]),
                in1=_v(tau3, 0, [[1, 3], [0, BND]]),
                op=ALU.subtract)
            nc.scalar.activation(hx3[:], hx3[:], ACTF.Abs)
            nc.scalar.activation(hx3[:], hx3[:], ACTF.Relu, bias=1.0, scale=-1.0)
            nc.vector.tensor_tensor(
                out=w3d[:], in0=w3d[:],
                in1=_v(hx3, 0, [[BND, 3], [0, 3], [1, BND]]),
                op=ALU.mult)
        pv = [pup.tile([128, 512], F32, tag="acc", bufs=4, name=f"pv{ch}")
              for ch in range(2)]
        for di in range(3):
            tmp3 = pool.tile([128, 3, 48, BND], BF16, tag="tmp3",
                             name="tmp3", bufs=2)
            with nc.allow_low_precision(reason="sampling taps bf16"):
                nc.vector.tensor_tensor(
                    out=tmp3[:, 0:2],
                    in0=_v(wvx2, 0, [[1, 2], [NYV, 48], [1, BND]]),
                    in1=_v(w3d, di * 3 * BND, [[BND, 2], [0, 48], [1, BND]]),
                    op=ALU.mult)
                nc.vector.tensor_tensor(
                    out=tmp3[:, 2],
                    in0=_v(wvx2, 2, [[NYV, 48], [1, BND]]),
                    in1=_v(w3d, di * 3 * BND + 2 * BND, [[0, 48], [1, BND]]),
                    op=ALU.mult)
            lhs = _v(s3m, di * 128, [[1, 128]])
            for ty in range(3):
                for ch in range(2):
                    nc.tensor.matmul(
                        pv[ch][:128, :384], lhs,
                        _v(tmp3, ty * 48 * BND + ch * 384, [[1, 384]]),
                        start=(di == 0 and ty == 0),
                        stop=(di == 2 and ty == 2))
        # evict + bias -> x-major f32 out [128, 48, 16]
        s3of = pool.tile([128, 48, BND], F32)
        for ch in range(2):
            nc.vector.tensor_tensor(
                out=_v(s3of, ch * 384, [[1, 384]]),
                in0=pv[ch][:128, :384],
                in1=_vp(b3r, 0, 128, ch * 24, [[1, 24], [0, BND]]),
                op=ALU.add)
            nc.sync.dma_start(out_d[:, ch * 24:(ch + 1) * 24],
                              _v(s3of, ch * 384, [[1, 384]]))
    nc.compile()
    return nc


def _hat_consts(inputs):
    fold_ln = not np.any(inputs['ln_b'])
    lnscale = (inputs['ln_w'].astype(np.float32) if fold_ln
               else np.ones(96, np.float32))
    w1 = inputs['w1'][:, :, 0, 0].astype(np.float32)
    b1 = inputs['b1'].astype(np.float32)
    ow2 = inputs['off_w2'].astype(np.float32)
    w2f = inputs['w2'][:, 0].reshape(288, 9).astype(np.float32)
    OW = np.zeros((18, 9, 49), np.float32)
    for ty in range(3):
        for tx in range(3):
            t = ty * 3 + tx
            OW[:, t, :48] = ow2[:, :, ty, tx] @ w1
            OW[:, t, 48] = ow2[:, :, ty, tx] @ b1
    A = np.zeros((288, 9, 49), np.float32)
    for k in range(9):
        A[:, k, :48] = w2f[:, k:k + 1] * w1
        A[:, k, 48] = w2f[:, k] * b1
    Afull = np.zeros((442, 288), np.float32)
    Afull[:441] = A.reshape(288, 441).T
    Afull[441] = inputs['b2'].astype(np.float32)
    a4 = np.zeros((128, 4, 288), np.float32)
    for c in range(3):
        a4[:, c, :] = Afull[CB[c]:CB[c] + 128]
    a4[:64, 3, :] = Afull[378:442]
    a4[:6, 3, :] = 0.0  # slots 378..383 already counted in chunk 2
    # block-diag pair DFT matrices
    TfRe = np.zeros((40, 64), np.float32)
    TfIm = np.zeros((40, 64), np.float32)
    basis = np.zeros((8, 8))
    for y in range(8):
        for x in range(8):
            basis[:] = 0.0
            basis[y, x] = 1.0
            Fz = np.fft.rfft2(basis)
            TfRe[:, y * 8 + x] = Fz.real.reshape(-1)
            TfIm[:, y * 8 + x] = Fz.imag.reshape(-1)
    TiR = np.zeros((64, 40), np.float32)
    TiI = np.zeros((64, 40), np.float32)
    for j in range(40):
        fy, fx = divmod(j, 5)
        Z = np.zeros((8, 5), np.complex64)
        Z[fy, fx] = 1.0
        TiR[:, j] = np.fft.irfft2(Z, s=(8, 8)).reshape(-1)
        Z[fy, fx] = 1j
        TiI[:, j] = np.fft.irfft2(Z, s=(8, 8)).reshape(-1)
    tf2Re = np.zeros((128, 80), np.float32)
    tf2Im = np.zeros((128, 80), np.float32)
    ti2Re = np.zeros((80, 128), np.float32)
    ti2Im = np.zeros((80, 128), np.float32)
    for h in range(2):
        tf2Re[64 * h:64 * h + 64, 40 * h:40 * h + 40] = TfRe.T
        tf2Im[64 * h:64 * h + 64, 40 * h:40 * h + 40] = TfIm.T
        ti2Re[40 * h:40 * h + 40, 64 * h:64 * h + 64] = TiR.T
        ti2Im[40 * h:40 * h + 40, 64 * h:64 * h + 64] = TiI.T
    # shift matrices: S7[xin, di, xout] = 1 iff xout == xin - (di-3)
    s7 = np.zeros((128, 7, 128), np.float32)
    for di in range(7):
        d = di - 3
        for xin in range(128):
            xo = xin - d
            if 0 <= xo < 128:
                s7[xin, di, xo] = 1.0
    s3m = np.ascontiguousarray(s7[:, 2:5, :])
    tau79 = np.zeros((7, 9), np.float32)
    for di in range(7):
        for k in range(9):
            t = (di - 3) - (k % 3) + 1
            tau79[di, k] = t if abs(t) <= 2 else 99.0
    owt = np.ascontiguousarray(OW.transpose(2, 1, 0))  # [49, 9, 18]

    def b16(a):
        return np.ascontiguousarray(a).astype(_bf16)

    return dict(
        fold_ln=fold_ln,
        a4=b16(a4), owt=b16(owt),
        tf2Re=b16(tf2Re), tf2Im=b16(tf2Im),
        ti2Re=b16(ti2Re), ti2Im=b16(ti2Im),
        identb=b16(np.eye(128, dtype=np.float32)),
        s7=b16(s7), s3m=b16(s3m),
        ow1T=b16(inputs['off_w1'][:, :, 0, 0].T.astype(np.float32)),
        tau3=b16(np.tile(np.array([-1, 0, 1], np.float32), (128, 1))),
        tau5=b16(np.tile(np.arange(-2, 3, dtype=np.float32), (128, 1))),
        tau79y=b16(np.tile(tau79.reshape(1, 7, 9, 1), (128, 1, 1, BND))),
        lnw=b16(np.tile(inputs['ln_w'].astype(np.float32), (128, 1))),
        lnb=b16(np.tile(inputs['ln_b'].astype(np.float32), (128, 1))),
        wo50=b16(np.concatenate(
            [(inputs['w3'][:, :, 0, 0] * lnscale[None, :]).T,
             (inputs['off_w3'][:, :, 0, 0] * lnscale[None, :]).T],
            axis=1).astype(np.float32)),
        b3r=np.tile(inputs['b3'].astype(np.float32)[None, :], (128, 1)),
    )


def make_in_maps(inputs):
    C = _hat_consts(inputs)
    x = np.asarray(inputs['x'][0], np.float32)
    xp = np.zeros((48, H + 8, W), np.float32)
    xp[:, 4:4 + H, :] = x
    xpb = xp.astype(_bf16)
    in_a = []
    for ci in range(NCORES):
        onesp = np.zeros((128, NYS), np.float32)
        for j in range(NYS):
            if 0 <= 16 * ci - 3 + j < H:
                onesp[:, j] = 1.0
        in_a.append(dict(
            xh=np.ascontiguousarray(xpb[:, 16 * ci + 1:16 * ci + 1 + NYS, :]),
            xt=np.ascontiguousarray(
                xpb[:, 16 * ci:16 * ci + NYX, :].transpose(2, 0, 1)),
            onesp=onesp.astype(_bf16),
            identb=C['identb'], ow1T=C['ow1T'], owt=C['owt'], a4=C['a4'],
            tf2Re=C['tf2Re'], tf2Im=C['tf2Im'], ti2Re=C['ti2Re'],
            ti2Im=C['ti2Im'], s7=C['s7'], tau3=C['tau3'], tau5=C['tau5'],
            tau79y=C['tau79y'], lnw=C['lnw'], lnb=C['lnb']))
    return C, in_a


def unscramble_vo(res_a):
    """[128, 16, 96] pixel-major per core -> [96, H, 128] channel-major."""
    parts = []
    for r in res_a:
        vp = r['vo_out'].reshape(2, 8, 8, 16, 96)   # [ph, py, px, pc, c]
        parts.append(vp.transpose(4, 0, 1, 3, 2).reshape(96, BND, 128))
    return np.concatenate(parts, axis=1)


def make_in_maps_b(C, vo_full):
    vop = np.zeros((96, H + 2, 128), _bf16)
    vop[:, 1:1 + H, :] = vo_full
    in_b = []
    for ci in range(NCORES):
        in_b.append(dict(
            voh=np.ascontiguousarray(vop[:, 16 * ci:16 * ci + NYV, :]),
            wo50=C['wo50'], b3r=C['b3r'], tau3=C['tau3'], s3m=C['s3m']))
    return in_b


_CACHE = {}


def kernel(**inputs):
    C, in_a = make_in_maps(inputs)
    key = 'nca_fold' if C['fold_ln'] else 'nca'
    if key not in _CACHE:
        _CACHE[key] = build_kernel_a(fold_ln=C['fold_ln'])
        _CACHE.setdefault('ncb', build_kernel_b())
    nca, ncb = _CACHE[key], _CACHE['ncb']
    res_a = bass_utils.run_bass_kernel_spmd(nca, in_a, core_ids=list(range(NCORES)))
    vo_full = unscramble_vo(res_a.results)
    in_b = make_in_maps_b(C, vo_full)
    res_b = bass_utils.run_bass_kernel_spmd(ncb, in_b, core_ids=list(range(NCORES)))
    # outp [128, 48, 16] x-major -> [48, 16, 128]
    out = np.concatenate([r['outp'].transpose(1, 2, 0) for r in res_b.results],
                         axis=1)
    return out[None].astype(np.float32)


# revision 58
# speedup vs baseline: 1.8186x; 1.0067x over previous
"""Trainium2 Bass kernel for nn_DeformAttn (deformable attention, patch-FFT).

Self-contained: hardcodes shapes for x [1,48,128,128], 8 NeuronCores,
y-band split (16 rows/core).

Design (per core):
- All deformable convs use dense hat-tap sampling with COLUMN-SHIFTED weight
  evaluation: for each x-shift delta, the 2D hat weights (hy*hx, both read
  from the delta-shifted offset field so they are evaluated at the OUTPUT
  pixel) premultiply the unshifted source on DVE (a slice of the work goes
  to the GpSimd/Pool engine); the x-shift-and-add runs on the PE as banded
  0/1 shift matmuls accumulating in PSUM, with the y-tap reduction riding
  the same PSUM accumulation (one matmul per ty plane).
- Layer-1 (1x1 deform, |off|<1): 3x3 taps.  Layer-2 (3x3 depthwise deform,
  |off|<2): 5x5 taps per kernel point, 7 global x-shifts.  Layer-3: 3x3 taps.
- qkv = A^T u with host-precomputed Khatri-Rao A (w2 (x) W1), bias via
  indicator/ones rows.  off2 via OW matrix on channel-major s1x.
- Hat weights are built on the UNSHIFTED offset field (one abs/relu pass)
  and then column-shifted by SBUF->SBUF DMAs (hy) / PE shift-matmuls
  (off1/off3); per-delta hat products chase the shift DMAs.
- x arrives in BOTH channel-major and x-major layouts from the host (no
  on-device input transposes).  vo leaves pixel-major; the host transposes.
- Patch FFT (8x8 circular conv): patch-PAIR block-diagonal real-DFT matmuls
  (128 partitions = 2 patches), complex pointwise on DVE, block-diag inverse.
- Everything bf16 on-chip except PSUM accumulation and LN statistics.
"""
import numpy as np
import ml_dtypes
_bf16 = np.float16
from contextlib import ExitStack

import concourse.bacc as bacc
import concourse.mybir as mybir
import concourse.tile as tile
from concourse.bass import AP
from concourse import bass_utils

dt = mybir.dt
F32 = dt.float32
BF16 = dt.float16
ALU = mybir.AluOpType
ACTF = mybir.ActivationFunctionType
AX = mybir.AxisListType

H = W = 128
BND = 16          # band rows per core
NYS = 22          # s1x rows per core (band +/-3)
NYX = 24          # x rows per core (band +/-4)
NYV = 18          # vo rows per core (band +/-1)
NCORES = 8
CB = [0, 128, 256, 314]   # uT K-chunk bases over 442 slots

def _v(t, off, dims):
    """View of tile t: keep its full partition dim, custom free dims."""
    return AP(t.tensor, t.offset + off, [list(t.ap[0])] + [list(d) for d in dims])


def _vp(t, p0, np_, off, dims):
    """View with partition sub-range [p0, p0+np_) and custom free dims."""
    st = t.ap[0][0]
    return AP(t.tensor, t.offset + p0 * st + off,
              [[st, np_]] + [list(d) for d in dims])


def build_kernel_a(fold_ln=False):
    nc = bacc.Bacc("TRN2", target_bir_lowering=False, debug=False)
    xh_d = nc.dram_tensor("xh", [48, NYS, W], BF16, kind="ExternalInput")
    xt_d = nc.dram_tensor("xt", [128, 48, NYX], BF16, kind="ExternalInput")
    onesp_d = nc.dram_tensor("onesp", [128, NYS], BF16, kind="ExternalInput")
    identb_d = nc.dram_tensor("identb", [128, 128], BF16, kind="ExternalInput")
    ow1T_d = nc.dram_tensor("ow1T", [48, 2], BF16, kind="ExternalInput")
    owt_d = nc.dram_tensor("owt", [49, 9, 18], BF16, kind="ExternalInput")
    a4_d = nc.dram_tensor("a4", [128, 4, 288], BF16, kind="ExternalInput")
    tf2Re_d = nc.dram_tensor("tf2Re", [128, 80], BF16, kind="ExternalInput")
    tf2Im_d = nc.dram_tensor("tf2Im", [128, 80], BF16, kind="ExternalInput")
    ti2Re_d = nc.dram_tensor("ti2Re", [80, 128], BF16, kind="ExternalInput")
    ti2Im_d = nc.dram_tensor("ti2Im", [80, 128], BF16, kind="ExternalInput")
    s7_d = nc.dram_tensor("s7", [128, 7, 128], BF16, kind="ExternalInput")
    tau3_d = nc.dram_tensor("tau3", [128, 3], BF16, kind="ExternalInput")
    tau5_d = nc.dram_tensor("tau5", [128, 5], BF16, kind="ExternalInput")
    tau79y_d = nc.dram_tensor("tau79y", [128, 7, 9, BND], BF16,
                              kind="ExternalInput")
    lnw_d = nc.dram_tensor("lnw", [128, 96], BF16, kind="ExternalInput")
    lnb_d = nc.dram_tensor("lnb", [128, 96], BF16, kind="ExternalInput")
    vo_out = nc.dram_tensor("vo_out", [128, BND, 96], BF16,
                            kind="ExternalOutput")

    with tile.TileContext(nc) as tc, ExitStack() as top:
        cpool = top.enter_context(tc.tile_pool(name="consts", bufs=1))
        xh = cpool.tile([48, NYS, W], BF16)
        xt = cpool.tile([128, 48, NYX], BF16)
        identb = cpool.tile([128, 128], BF16)
        ow1T = cpool.tile([48, 2], BF16)
        owt = cpool.tile([49, 9, 18], BF16)
        a4 = cpool.tile([128, 4, 288], BF16)
        tf2Re = cpool.tile([128, 80], BF16)
        tf2Im = cpool.tile([128, 80], BF16)
        ti2Re = cpool.tile([80, 128], BF16)
        ti2Im = cpool.tile([80, 128], BF16)
        s7 = cpool.tile([128, 7, 128], BF16)
        tau3 = cpool.tile([128, 3], BF16)
        tau5 = cpool.tile([128, 5], BF16)
        tau79y = cpool.tile([128, 7, 9, BND], BF16)
        lnw = cpool.tile([128, 96], BF16)
        lnb = cpool.tile([128, 96], BF16)
        onesp = cpool.tile([128, NYS], BF16, name="onesp")
        # one queue (the model serializes HWDGE anyway; issuing from Act
        # stalls the Act sequencer), ordered by first use
        first = [(xh, xh_d), (ow1T, ow1T_d), (identb, identb_d),
                 (s7, s7_d), (xt, xt_d),
                 (tau3, tau3_d), (onesp, onesp_d),
                 (owt, owt_d), (tau5, tau5_d), (tau79y, tau79y_d),
                 (a4, a4_d), (tf2Re, tf2Re_d), (tf2Im, tf2Im_d),
                 (ti2Re, ti2Re_d), (ti2Im, ti2Im_d)]
        if not fold_ln:
            first += [(lnw, lnw_d), (lnb, lnb_d)]
        for sb, dr in first:
            nc.sync.dma_start(sb[:], dr[:])

        scr = cpool.tile([1, 8], F32, name="scr")
        nc.vector.memset(scr[:], 1.0)
        nc.scalar.activation(scr[:], scr[:], ACTF.Sqrt)

        psum = top.enter_context(tc.tile_pool(name="psum", bufs=3, space="PSUM"))
        pupool = top.enter_context(tc.tile_pool(name="pu", bufs=4, space="PSUM"))

        big_cm = tc.tile_pool(name="big", bufs=1)
        bpool = big_cm.__enter__()
        s1x = bpool.tile([128, 49, NYS], BF16)
        u = bpool.tile([128, 442, BND], BF16)
        uT = bpool.tile([128, 4, 16, 128], BF16)
        qkv = bpool.tile([128, 16, 288], BF16)

        # ================= phase X: off1, layer-1 ==========================
        with tc.tile_pool(name="px", bufs=1) as p1:
            # off1 on s1x rows -> one psum, region-accumulated
            po = pupool.tile([128, 512], F32, tag="acc", name="po", bufs=3)
            for y in range(NYS):
                nc.tensor.matmul(po[:128, 2 * y:2 * y + 2],
                                 _v(xh, y * W, [[1, 128]]),
                                 ow1T[:], start=True, stop=True)
            off1pm = p1.tile([128, NYS, 2], BF16)
            nc.scalar.copy(off1pm[:], po[:128, :44])
            # shifted offset copies via PE shift-matmuls (edges auto-zero):
            # slot d=-1 (di=0): w[xin] = off[xin+1]; slot d=+1: off[xin-1]
            off1s3 = p1.tile([128, 3, NYS, 2], BF16)
            nc.vector.tensor_copy(off1s3[:, 1], off1pm[:])
            po1s = pupool.tile([128, 512], F32, tag="acc", name="po1s", bufs=3)
            nc.tensor.matmul(po1s[:128, 0:44], _v(s7, 4 * 128, [[1, 128]]),
                             _v(off1pm, 0, [[1, 44]]), start=True, stop=True)
            nc.tensor.matmul(po1s[:128, 44:88], _v(s7, 2 * 128, [[1, 128]]),
                             _v(off1pm, 0, [[1, 44]]), start=True, stop=True)
            nc.scalar.copy(_v(off1s3, 0, [[2 * 2 * NYS, 2], [1, 44]]),
                           po1s[:128, :88])
            # warm the PE p-state while W1d/tmp1 are being built so the
            # layer-1 matmuls run at full clock (junk transposes, never read)
            jp = psum.tile([128, 1024], BF16, tag="psb", name="jp", bufs=3)
            for _ in range(38):
                nc.tensor.transpose(jp[:128, :128], identb[:, :], identb[:, :])
            # W1d [128, 3d, 3ty, 22] = hat(oy_sh - (ty-1)) * hat(ox_sh - d)
            w1d = p1.tile([128, 3, 3, NYS], BF16)
            hx1 = p1.tile([128, 3, NYS], BF16)
            with nc.allow_low_precision(reason="hat weights bf16"):
                nc.vector.tensor_tensor(
                    out=w1d[:],
                    in0=_v(off1s3, 0, [[2 * NYS, 3], [0, 3], [2, NYS]]),
                    in1=_v(tau3, 0, [[0, 3], [1, 3], [0, NYS]]),
                    op=ALU.subtract)
                nc.scalar.activation(w1d[:], w1d[:], ACTF.Abs)
                nc.scalar.activation(w1d[:], w1d[:], ACTF.Relu,
                                     bias=1.0, scale=-1.0)
                nc.vector.tensor_tensor(
                    out=hx1[:],
                    in0=_v(off1s3, 1, [[2 * NYS, 3], [2, NYS]]),
                    in1=_v(tau3, 0, [[1, 3], [0, NYS]]),
                    op=ALU.subtract)
                nc.scalar.activation(hx1[:], hx1[:], ACTF.Abs)
                nc.scalar.activation(hx1[:], hx1[:], ACTF.Relu,
                                     bias=1.0, scale=-1.0)
                nc.vector.tensor_tensor(
                    out=w1d[:], in0=w1d[:],
                    in1=_v(hx1, 0, [[NYS, 3], [0, 3], [1, NYS]]),
                    op=ALU.mult)
            # layer-1 sampling: premult per delta, PE shift-accumulate
            ps1 = [pupool.tile([128, 512], F32, tag="acc", bufs=3, name=f"ps1_{c}")
                   for c in range(3)]
            tmp1s = []
            for di in range(3):
                tmp1 = p1.tile([128, 3, 48, NYS], BF16, tag="tmp1",
                               name="tmp1", bufs=3)
                with nc.allow_low_precision(reason="sampling taps bf16"):
                    nc.vector.tensor_tensor(
                        out=tmp1[:],
                        in0=_v(xt, 0, [[1, 3], [NYX, 48], [1, NYS]]),
                        in1=_v(w1d, di * 3 * NYS, [[NYS, 3], [0, 48], [1, NYS]]),
                        op=ALU.mult)
                tmp1s.append(tmp1)
            for di in range(3):
                tmp1 = tmp1s[di]
                lhs = _v(s7, (di + 2) * 128, [[1, 128]])  # delta=-1,0,1 -> slots 2,3,4
                for ty in range(3):
                    for ch in range(3):
                        nc.tensor.matmul(
                            ps1[ch][:128, :352], lhs,
                            _v(tmp1, ty * 48 * NYS + ch * 352, [[1, 352]]),
                            start=(di == 0 and ty == 0),
                            stop=(di == 2 and ty == 2))
            for ch in range(3):
                (nc.scalar.copy if ch != 1 else nc.vector.tensor_copy)(
                    _v(s1x, ch * 352, [[1, 352]]), ps1[ch][:128, :352])
            nc.sync.dma_start(_v(s1x, 48 * NYS, [[1, NYS]]), onesp[:])

        # ================= phase O: s1xT, off2, W2d =========================
        w2ds = [bpool.tile([128, 9, 5, BND], BF16, name=f"w2d{di}")
                for di in range(7)]
        with tc.tile_pool(name="po2", bufs=1) as p2:
            hox7 = p2.tile([128, 7, 864], BF16)
            nc.gpsimd.memset(hox7[:], 0.0)
            s1xT = p2.tile([49, NYS, 130], BF16)
            nc.gpsimd.memset(_v(s1xT, 0, [[130, NYS], [1, 1]]), 0.0)
            nc.gpsimd.memset(_v(s1xT, 129, [[130, NYS], [1, 1]]), 0.0)
            for gi, (g0, gn) in enumerate([(0, 8), (8, 8), (16, 6)]):
                ps = psum.tile([128, 1024], BF16, tag="psb", name="ps", bufs=3)
                for i in range(gn):
                    nc.tensor.transpose(ps[:49, i * 128:(i + 1) * 128],
                                        _v(s1x, g0 + i, [[NYS, 49]]),
                                        identb[:, :])
                dst = _v(s1xT, g0 * 130 + 1, [[130, gn], [1, 128]])
                (nc.scalar.copy if gi % 2 == 0 else nc.vector.tensor_copy)(
                    dst, ps[:49, :gn * 128])
            # off2: per band row b, 9 taps accumulate; 2 psum region-tiles
            pofs = [pupool.tile([128, 512], F32, tag="acc", bufs=3, name=f"po2_{h}")
                    for h in range(2)]
            for b in range(BND):
                po2 = pofs[b // 8]
                col = 18 * (b % 8)
                for t in range(9):
                    ty, tx = divmod(t, 3)
                    nc.tensor.matmul(
                        po2[:128, col:col + 18],
                        _v(s1xT, (b + 2 + ty) * 130 + tx, [[1, 128]]),
                        owt[:, t], start=(t == 0), stop=(t == 8))
            off2pm = p2.tile([128, BND, 18], BF16)
            nc.scalar.copy(_v(off2pm, 0, [[1, 144]]), pofs[0][:128, :144])
            nc.scalar.copy(_v(off2pm, 144, [[1, 144]]), pofs[1][:128, :144])
            # base hat_y + compact ox on the UNSHIFTED field, side by side:
            # hob = [hyb (720) | oxb (144)]
            hob = p2.tile([128, 864], BF16)
            with nc.allow_low_precision(reason="hat weights bf16"):
                nc.vector.tensor_tensor(
                    out=_v(hob, 0, [[80, 9], [16, 5], [1, BND]]),
                    in0=_v(off2pm, 0, [[2, 9], [0, 5], [18, BND]]),
                    in1=_v(tau5, 0, [[0, 9], [1, 5], [0, BND]]),
                    op=ALU.subtract)
                nc.vector.tensor_copy(
                    _v(hob, 720, [[16, 9], [1, BND]]),
                    _v(off2pm, 1, [[2, 9], [18, BND]]))
                nc.scalar.activation(_v(hob, 0, [[1, 720]]),
                                     _v(hob, 0, [[1, 720]]), ACTF.Abs)
                nc.scalar.activation(_v(hob, 0, [[1, 720]]),
                                     _v(hob, 0, [[1, 720]]), ACTF.Relu,
                                     bias=1.0, scale=-1.0)
            # column-shifted copies hox7[xin, di] = hob[xin - d] via DMAs;
            # edge partitions stay zero from the early Pool memset.  hxb and
            # the w2d slice for each di chase its shift DMA.
            hxb = p2.tile([128, 7, 9, BND], BF16)
            for d in range(-3, 4):
                di = d + 3
                if d > 0:
                    nc.sync.dma_start(hox7[d:128, di], hob[0:128 - d])
                elif d < 0:
                    nc.sync.dma_start(hox7[0:128 + d, di], hob[-d:128])
                else:
                    nc.sync.dma_start(hox7[:, di], hob[:])
                hxd = _v(hxb, di * 9 * BND, [[BND, 9], [1, BND]])
                with nc.allow_low_precision(reason="hat weights bf16"):
                    nc.gpsimd.tensor_tensor(
                        out=hxd,
                        in0=_v(hox7, di * 864 + 720, [[16, 9], [1, BND]]),
                        in1=_v(tau79y, di * 9 * BND, [[BND, 9], [1, BND]]),
                        op=ALU.subtract)
                    nc.scalar.activation(hxd, hxd, ACTF.Abs)
                    nc.scalar.activation(hxd, hxd, ACTF.Relu,
                                         bias=1.0, scale=-1.0)
                    nc.vector.tensor_tensor(
                        out=w2ds[di][:],
                        in0=_v(hox7, di * 864, [[80, 9], [16, 5], [1, BND]]),
                        in1=_v(hxb, di * 9 * BND, [[BND, 9], [0, 5], [1, BND]]),
                        op=ALU.mult)

        # keep the PE p-state warm across the hox7 shift-DMA stall so the
        # first sampling groups run at full clock
        jp2 = psum.tile([128, 1024], BF16, tag="psb", name="jp2", bufs=3)
        for _ in range(48):
            nc.tensor.transpose(jp2[:128, :128], identb[:, :], identb[:, :])

        # ========== phase S: sampling (pipelined premults, uT dribbled) ====
        # Premults for group k+1 (DVE) / k+2 (Pool) issue while the PE
        # consumes group k's 50 stall-free matmuls; uT transposes fill the
        # inter-group gaps so the PE p-state stays ramped.
        DVE_S = [0, 1, 3, 4]
        MM_S = [0, 1, 3, 4, 2]

        def emit_uT_group(c, g, eng):
            # c 0..2: full 128-slot chunks; c 3: slots 378..441 (64; slots
            # 378..383 are zeroed in a4 since chunk 2 already counts them)
            base, nsl = (CB[c], 128) if c < 3 else (378, 64)
            ps = psum.tile([128, 1024], BF16, tag="psb", name="ps", bufs=3)
            for yy in range(8):
                y = g * 8 + yy
                nc.tensor.transpose(
                    ps[:nsl, yy * 128:(yy + 1) * 128],
                    _v(u, base * BND + y, [[BND, nsl]]),
                    identb[:, :])
            dst = _vp(uT, 0, nsl, c * 2048 + g * 64,
                      [[8, 8], [128, 16], [1, 8]])
            (nc.scalar.copy if eng == 0 else nc.vector.tensor_copy)(
                dst, ps[:nsl, :1024])

        with tc.tile_pool(name="psmp", bufs=1) as p3:
            tmps = {}

            def emit_pool_premult(k):
                ki, kj = divmod(k, 3)
                tmp = p3.tile([128, 5, 49, BND], BF16, tag="tmpp",
                              name="tmpp", bufs=3)
                with nc.allow_low_precision(reason="sampling taps bf16"):
                    nc.gpsimd.tensor_tensor(
                        out=tmp[:],
                        in0=_v(s1x, ki, [[1, 5], [NYS, 49], [1, BND]]),
                        in1=_v(w2ds[kj + 2], k * 5 * BND,
                               [[BND, 5], [0, 49], [1, BND]]),
                        op=ALU.mult)
                tmps[(k, 2)] = tmp

            def emit_dve_premults(k):
                ki, kj = divmod(k, 3)
                for s in DVE_S:
                    tmp = p3.tile([128, 5, 49, BND], BF16, tag="tmp",
                                  name="tmp", bufs=10)
                    with nc.allow_low_precision(reason="sampling taps bf16"):
                        nc.vector.tensor_tensor(
                            out=tmp[:],
                            in0=_v(s1x, ki, [[1, 5], [NYS, 49], [1, BND]]),
                            in1=_v(w2ds[kj + s], k * 5 * BND,
                                   [[BND, 5], [0, 49], [1, BND]]),
                            op=ALU.mult)
                    tmps[(k, s)] = tmp

            # (chunk, group) transposes dribbled after group k's matmuls
            UT_SCHED = {3: [(0, 0)], 4: [(0, 1)], 6: [(1, 0)], 7: [(1, 1)],
                        8: [(2, 0), (2, 1)]}

            emit_pool_premult(0)
            emit_pool_premult(1)
            emit_dve_premults(0)
            for k in range(9):
                ki, kj = divmod(k, 3)
                if k < 8:
                    emit_dve_premults(k + 1)
                if k < 7:
                    emit_pool_premult(k + 2)
                if k < 8:
                    for c, g in UT_SCHED.get(k, []):
                        emit_uT_group(c, g, 0)
                pk = [pupool.tile([128, 512], F32, tag="acc", bufs=3,
                                  name=f"pk{ch}") for ch in range(2)]
                mm_order = [0, 1, 2, 3, 4] if k == 8 else MM_S
                for si, s in enumerate(mm_order):
                    di = kj + s
                    tmp = tmps.pop((k, s))
                    lhs = _v(s7, di * 128, [[1, 128]])
                    for ty in range(5):
                        for ch in range(2):
                            nc.tensor.matmul(
                                pk[ch][:128, :392], lhs,
                                _v(tmp, ty * 784 + ch * 392, [[1, 392]]),
                                start=(si == 0 and ty == 0),
                                stop=(si == 4 and ty == 4))
                nc.scalar.copy(_v(u, k * 784, [[1, 392]]),
                               pk[0][:128, :392])
                (nc.vector.tensor_copy if k == 8 else nc.scalar.copy)(
                    _v(u, k * 784 + 392, [[1, 392]]), pk[1][:128, :392])
                if k == 8:
                    nc.vector.memset(_v(u, 441 * BND, [[1, BND]]), 1.0)
                    for ci, (c, g) in enumerate(UT_SCHED[8]):
                        emit_uT_group(c, g, ci % 2)

        # ================= phase Q + FFT ====================================
        with tc.tile_pool(name="pfft", bufs=1) as fp:
            qhRe = fp.tile([80, 16, 192], BF16)
            qhIm = fp.tile([80, 16, 192], BF16)
            for g in range(2):
                emit_uT_group(3, g, g % 2)
            for pc in range(16):
                qp = pupool.tile([128, 512], F32, tag="acc", bufs=3,
                                 name="qp")
                for c in range(3):
                    nc.tensor.matmul(qp[:128, :288],
                                     _v(uT, c * 2048 + pc * 128, [[1, 128]]),
                                     a4[:, c], start=(c == 0), stop=False)
                nc.tensor.matmul(qp[:128, :288],
                                 _vp(uT, 0, 64, 3 * 2048 + pc * 128,
                                     [[1, 128]]),
                                 _vp(a4, 0, 64, 3 * 288, [[1, 288]]),
                                 start=False, stop=True)
                if pc % 2 == 0:
                    nc.scalar.copy(_v(qkv, pc * 288, [[1, 288]]),
                                   qp[:128, :288])
                else:
                    nc.vector.tensor_copy(_v(qkv, pc * 288, [[1, 288]]),
                                          qp[:128, :288])
                if pc % 2 == 1:
                    g = pc // 2
                    rhs = _v(qkv, 2 * g * 288, [[288, 2], [1, 192]])
                    psR = psum.tile([128, 512], F32, tag="ps", name="ps",
                                    bufs=2)
                    nc.tensor.matmul(psR[:80, :384], tf2Re[:], rhs,
                                     start=True, stop=True)
                    nc.scalar.copy(_v(qhRe, 2 * g * 192, [[1, 384]]),
                                   psR[:80, :384])
                    psI = psum.tile([128, 512], F32, tag="ps", name="ps",
                                    bufs=2)
                    nc.tensor.matmul(psI[:80, :384], tf2Im[:], rhs,
                                     start=True, stop=True)
                    (nc.vector.tensor_copy if g % 2 == 0 else nc.scalar.copy)(
                        _v(qhIm, 2 * g * 192, [[1, 384]]), psI[:80, :384])
            # complex pointwise + inverse DFT + LN partial stats, pipelined
            # per quarter (4 patch-pairs each)
            ohRe = fp.tile([80, 16, 96], BF16)
            ohIm = fp.tile([80, 16, 96], BF16)
            t1 = fp.tile([80, 16, 96], BF16)
            t2 = fp.tile([80, 16, 96], BF16)
            t1b = fp.tile([80, 16, 96], BF16)
            t2b = fp.tile([80, 16, 96], BF16)
            osb = fp.tile([128, 16, 96], BF16)
            ssum = fp.tile([128, BND], F32)
            sqs = fp.tile([128, BND], F32)
            sq = fp.tile([128, 16, 96], BF16)
            with nc.allow_low_precision(reason="fft products bf16"):
                for q in range(4):
                    o = q * 4 * 192
                    arq = _v(qhRe, o, [[192, 4], [1, 96]])
                    brq = _v(qhRe, o + 96, [[192, 4], [1, 96]])
                    aiq = _v(qhIm, o, [[192, 4], [1, 96]])
                    biq = _v(qhIm, o + 96, [[192, 4], [1, 96]])
                    oq = q * 384
                    t1q = _v(t1, oq, [[1, 384]])
                    t2q = _v(t2, oq, [[1, 384]])
                    t1bq = _v(t1b, oq, [[1, 384]])
                    t2bq = _v(t2b, oq, [[1, 384]])
                    nc.vector.tensor_tensor(out=t1q, in0=arq, in1=brq,
                                            op=ALU.mult)
                    nc.gpsimd.tensor_tensor(out=t2q, in0=aiq, in1=biq,
                                            op=ALU.mult)
                    nc.vector.tensor_tensor(out=_v(ohRe, oq, [[1, 384]]),
                                            in0=t1q, in1=t2q, op=ALU.subtract)
                    nc.gpsimd.tensor_tensor(out=t2bq, in0=aiq, in1=brq,
                                            op=ALU.mult)
                    nc.vector.tensor_tensor(out=t1bq, in0=arq, in1=biq,
                                            op=ALU.mult)
                    nc.vector.tensor_tensor(out=_v(ohIm, oq, [[1, 384]]),
                                            in0=t1bq, in1=t2bq, op=ALU.add)
                    ps = psum.tile([128, 512], F32, tag="ps", name="ps", bufs=2)
                    nc.tensor.matmul(ps[:128, :384], ti2Re[:],
                                     _v(ohRe, oq, [[1, 384]]),
                                     start=True, stop=False)
                    nc.tensor.matmul(ps[:128, :384], ti2Im[:],
                                     _v(ohIm, oq, [[1, 384]]),
                                     start=False, stop=True)
                    (nc.scalar.copy if q % 2 == 0 else nc.vector.tensor_copy)(
                        _v(osb, oq, [[1, 384]]), ps[:128, :384])
                    nc.vector.tensor_reduce(
                        out=_v(ssum, q * 4, [[1, 4]]),
                        in_=_v(osb, oq, [[96, 4], [1, 96]]),
                        axis=AX.X, op=ALU.add)
                    nc.scalar.activation(_v(sq, oq, [[1, 384]]),
                                         _v(osb, oq, [[1, 384]]), ACTF.Square)
                    nc.vector.tensor_reduce(
                        out=_v(sqs, q * 4, [[1, 4]]),
                        in_=_v(sq, oq, [[96, 4], [1, 96]]),
                        axis=AX.X, op=ALU.add)
            # LayerNorm stats -> normalize -> gate by v, half at a time
            mu = fp.tile([128, BND], F32)
            var = fp.tile([128, BND], F32)
            musq = fp.tile([128, BND], F32)
            mub = fp.tile([128, BND, 96], BF16)
            invb = fp.tile([128, BND, 96], BF16)
            on = fp.tile([128, 16, 96], BF16)
            vo = fp.tile([128, 16, 96], BF16)
            nc.scalar.activation(mu[:], ssum[:], ACTF.Copy, scale=1.0 / 96)
            nc.scalar.activation(var[:], sqs[:], ACTF.Copy, scale=1.0 / 96)
            nc.scalar.activation(musq[:], mu[:], ACTF.Square)
            nc.vector.tensor_tensor(out=var[:], in0=var[:], in1=musq[:],
                                    op=ALU.subtract)
            std = fp.tile([128, BND], F32)
            inv = fp.tile([128, BND], F32)
            nc.vector.tensor_scalar_add(out=var[:], in0=var[:], scalar1=1e-5)
            nc.scalar.activation(std[:], var[:], ACTF.Sqrt)
            nc.vector.reciprocal(inv[:], std[:])
            nc.gpsimd.tensor_copy(mub[:], _v(mu, 0, [[1, BND], [0, 96]]))
            nc.scalar.copy(invb[:], _v(inv, 0, [[1, BND], [0, 96]]))
            with nc.allow_low_precision(reason="ln bf16"):
                for h in range(2):
                    oh = h * 8 * 96
                    onh = _v(on, oh, [[1, 768]])
                    nc.vector.tensor_tensor(
                        out=onh, in0=_v(osb, oh, [[1, 768]]),
                        in1=_v(mub, oh, [[1, 768]]),
                        op=ALU.subtract)
                    nc.vector.tensor_tensor(
                        out=onh, in0=onh,
                        in1=_v(invb, oh, [[1, 768]]),
                        op=ALU.mult)
                    if not fold_ln:
                        nc.vector.tensor_tensor(
                            out=onh, in0=onh,
                            in1=_v(lnw, 0, [[0, 8], [1, 96]]), op=ALU.mult)
                        nc.vector.tensor_tensor(
                            out=onh, in0=onh,
                            in1=_v(lnb, 0, [[0, 8], [1, 96]]), op=ALU.add)
                    nc.vector.tensor_tensor(
                        out=_v(vo, oh, [[1, 768]]), in0=onh,
                        in1=_v(qkv, 192 + h * 8 * 288, [[288, 8], [1, 96]]),
                        op=ALU.mult)
                    nc.sync.dma_start(vo_out[:, 8 * h:8 * (h + 1)],
                                      _v(vo, oh, [[1, 768]]))
        big_cm.__exit__(None, None, None)
    nc.compile()
    return nc


def build_kernel_b():
    """Layer-3: out = W3 @ deform_sample(vo) + b3 == deform_sample(W3 @ vo) + b3
    (sampling offsets are channel-shared, so the contraction commutes).
    W3-contraction and off3 ride ONE fused per-row matmul; output leaves
    x-major (host transposes back)."""
    nc = bacc.Bacc("TRN2", target_bir_lowering=False, debug=False)
    voh_d = nc.dram_tensor("voh", [96, NYV, 128], BF16, kind="ExternalInput")
    wo50_d = nc.dram_tensor("wo50", [96, 50], BF16, kind="ExternalInput")
    b3r_d = nc.dram_tensor("b3r", [128, 48], F32, kind="ExternalInput")
    tau3_d = nc.dram_tensor("tau3", [128, 3], BF16, kind="ExternalInput")
    s3_d = nc.dram_tensor("s3m", [128, 3, 128], BF16, kind="ExternalInput")
    out_d = nc.dram_tensor("outp", [128, 48, BND], F32, kind="ExternalOutput")
    with tile.TileContext(nc) as tc, ExitStack() as top:
        pool = top.enter_context(tc.tile_pool(name="b", bufs=1))
        pup = top.enter_context(tc.tile_pool(name="pub", bufs=4, space="PSUM"))
        voh = pool.tile([96, NYV, 128], BF16)
        wo50 = pool.tile([96, 50], BF16)
        b3r = pool.tile([128, 48], F32)
        tau3 = pool.tile([128, 3], BF16)
        s3m = pool.tile([128, 3, 128], BF16)
        nc.sync.dma_start(wo50[:], wo50_d[:])
        for rc in range(3):
            nc.sync.dma_start(voh[:, 6 * rc:6 * (rc + 1)],
                              voh_d[:, 6 * rc:6 * (rc + 1)])
        for sb, dr in [(tau3, tau3_d), (s3m, s3_d), (b3r, b3r_d)]:
            nc.sync.dma_start(sb[:], dr[:])
        # fused [W3 | ow3] per-row contraction -> wvx2 [128, 50, 18] c-major
        wvx2 = pool.tile([128, 50, NYV], BF16)
        pws = [pup.tile([128, 512], F32, tag="acc", bufs=4, name=f"pw{h}")
               for h in range(2)]
        for y in range(NYV):
            nc.tensor.matmul(pws[y // 9][:128, (y % 9) * 50:(y % 9) * 50 + 50],
                             _v(voh, y * 128, [[1, 128]]),
                             wo50[:], start=True, stop=True)
        for h in range(2):
            nc.scalar.copy(
                _v(wvx2, 48 * NYV + h * 9, [[1, 9], [NYV, 2]]),
                _v(pws[h], 48, [[50, 9], [1, 2]]))
        for h in range(2):
            (nc.scalar.copy if h == 0 else nc.vector.tensor_copy)(
                _v(wvx2, h * 9, [[1, 9], [NYV, 48]]),
                _v(pws[h], 0, [[50, 9], [1, 48]]))
        # off3 values live at wvx2 cols 48:50: off3[x, o, y] = wvx2[x, 48+o, y]
        off3s3 = pool.tile([128, 3, BND, 2], BF16)
        # center (d=0): off3s3[:, 1, b, o] = off3[x, o, b+1]
        nc.vector.tensor_copy(
            off3s3[:, 1], _v(wvx2, 48 * NYV + 1, [[1, BND], [NYV, 2]]))
        # shifted slots via PE shift-matmuls (edges auto-zero):
        # slot 0 (d=-1): off[xin+1] -> s3m slot 2; slot 2 (d=+1): slot 0
        p3s = pup.tile([128, 512], F32, tag="acc", name="p3s", bufs=4)
        nc.tensor.matmul(p3s[:128, 0:32], _v(s3m, 2 * 128, [[1, 128]]),
                         _v(off3s3, 32, [[1, 32]]), start=True, stop=True)
        nc.tensor.matmul(p3s[:128, 32:64], _v(s3m, 0, [[1, 128]]),
                         _v(off3s3, 32, [[1, 32]]), start=True, stop=True)
        nc.scalar.copy(_v(off3s3, 0, [[64, 2], [1, 32]]), p3s[:128, :64])
        # warm the PE p-state while the hat weights are built (s3m slot 1
        # is the identity matrix; junk transposes, never read)
        jb = pup.tile([128, 1024], BF16, tag="jnk", name="jb", bufs=1)
        for _ in range(34):
            nc.tensor.transpose(jb[:128, :128], _v(s3m, 128, [[1, 128]]),
                                _v(s3m, 128, [[1, 128]]))
        w3d = pool.tile([128, 3, 3, BND], BF16)
        hx3 = pool.tile([128, 3, BND], BF16)
        with nc.allow_low_precision(reason="hat weights bf16"):
            nc.vector.tensor_tensor(
                out=w3d[:],
                in0=_v(off3s3, 0, [[2 * BND, 3], [0, 3], [2, BND]]),
                in1=_v(tau3, 0, [[0, 3], [1, 3], [0, BND]]),
                op=ALU.subtract)
            nc.scalar.activation(w3d[:], w3d[:], ACTF.Abs)
            nc.scalar.activation(w3d[:], w3d[:], ACTF.Relu, bias=1.0, scale=-1.0)
            nc.vector.tensor_tensor(
                out=hx3[:],
                in0=_v(off3s3, 1, [[2 * BND, 3], [2, BND]]),
                in1=_v(tau3, 0, [[1, 3], [0, BND]]),
                op=ALU.subtract)
            nc.scalar.activation(hx3[:], hx3[:], ACTF.Abs)
            nc.scalar.activation(hx3[:], hx3[:], ACTF.Relu, bias=1.0, scale=-1.0)
            nc.vector.tensor_tensor(
                out=w3d[:], in0=w3d[:],
                in1=_v(hx3, 0, [[BND, 3], [0, 3], [1, BND]]),
                op=ALU.mult)
        pv = [pup.tile([128, 512], F32, tag="acc", bufs=4, name=f"pv{ch}")
              for ch in range(2)]
        for di in range(3):
            tmp3 = pool.tile([128, 3, 48, BND], BF16, tag="tmp3",
                             name="tmp3", bufs=2)
            with nc.allow_low_precision(reason="sampling taps bf16"):
                nc.vector.tensor_tensor(
                    out=tmp3[:, 0:2],
                    in0=_v(wvx2, 0, [[1, 2], [NYV, 48], [1, BND]]),
                    in1=_v(w3d, di * 3 * BND, [[BND, 2], [0, 48], [1, BND]]),
                    op=ALU.mult)
                nc.vector.tensor_tensor(
                    out=tmp3[:, 2],
                    in0=_v(wvx2, 2, [[NYV, 48], [1, BND]]),
                    in1=_v(w3d, di * 3 * BND + 2 * BND, [[0, 48], [1, BND]]),
                    op=ALU.mult)
            lhs = _v(s3m, di * 128, [[1, 128]])
            for ty in range(3):
                for ch in range(2):
                    nc.tensor.matmul(
                        pv[ch][:128, :384], lhs,
                        _v(tmp3, ty * 48 * BND + ch * 384, [[1, 384]]),
                        start=(di == 0 and ty == 0),
                        stop=(di == 2 and ty == 2))
        # evict + bias -> x-major f32 out [128, 48, 16]
        s3of = pool.tile([128, 48, BND], F32)
        for ch in range(2):
            nc.vector.tensor_tensor(
                out=_v(s3of, ch * 384, [[1, 384]]),
                in0=pv[ch][:128, :384],
                in1=_vp(b3r, 0, 128, ch * 24, [[1, 24], [0, BND]]),
                op=ALU.add)
            nc.sync.dma_start(out_d[:, ch * 24:(ch + 1) * 24],
                              _v(s3of, ch * 384, [[1, 384]]))
    nc.compile()
    return nc


def _hat_consts(inputs):
    fold_ln = not np.any(inputs['ln_b'])
    lnscale = (inputs['ln_w'].astype(np.float32) if fold_ln
               else np.ones(96, np.float32))
    w1 = inputs['w1'][:, :, 0, 0].astype(np.float32)
    b1 = inputs['b1'].astype(np.float32)
    ow2 = inputs['off_w2'].astype(np.float32)
    w2f = inputs['w2'][:, 0].reshape(288, 9).astype(np.float32)
    OW = np.zeros((18, 9, 49), np.float32)
    for ty in range(3):
        for tx in range(3):
            t = ty * 3 + tx
            OW[:, t, :48] = ow2[:, :, ty, tx] @ w1
            OW[:, t, 48] = ow2[:, :, ty, tx] @ b1
    A = np.zeros((288, 9, 49), np.float32)
    for k in range(9):
        A[:, k, :48] = w2f[:, k:k + 1] * w1
        A[:, k, 48] = w2f[:, k] * b1
    Afull = np.zeros((442, 288), np.float32)
    Afull[:441] = A.reshape(288, 441).T
    Afull[441] = inputs['b2'].astype(np.float32)
    a4 = np.zeros((128, 4, 288), np.float32)
    for c in range(3):
        a4[:, c, :] = Afull[CB[c]:CB[c] + 128]
    a4[:64, 3, :] = Afull[378:442]
    a4[:6, 3, :] = 0.0  # slots 378..383 already counted in chunk 2
    # block-diag pair DFT matrices
    TfRe = np.zeros((40, 64), np.float32)
    TfIm = np.zeros((40, 64), np.float32)
    basis = np.zeros((8, 8))
    for y in range(8):
        for x in range(8):
            basis[:] = 0.0
            basis[y, x] = 1.0
            Fz = np.fft.rfft2(basis)
            TfRe[:, y * 8 + x] = Fz.real.reshape(-1)
            TfIm[:, y * 8 + x] = Fz.imag.reshape(-1)
    TiR = np.zeros((64, 40), np.float32)
    TiI = np.zeros((64, 40), np.float32)
    for j in range(40):
        fy, fx = divmod(j, 5)
        Z = np.zeros((8, 5), np.complex64)
        Z[fy, fx] = 1.0
        TiR[:, j] = np.fft.irfft2(Z, s=(8, 8)).reshape(-1)
        Z[fy, fx] = 1j
        TiI[:, j] = np.fft.irfft2(Z, s=(8, 8)).reshape(-1)
    tf2Re = np.zeros((128, 80), np.float32)
    tf2Im = np.zeros((128, 80), np.float32)
    ti2Re = np.zeros((80, 128), np.float32)
    ti2Im = np.zeros((80, 128), np.float32)
    for h in range(2):
        tf2Re[64 * h:64 * h + 64, 40 * h:40 * h + 40] = TfRe.T
        tf2Im[64 * h:64 * h + 64, 40 * h:40 * h + 40] = TfIm.T
        ti2Re[40 * h:40 * h + 40, 64 * h:64 * h + 64] = TiR.T
        ti2Im[40 * h:40 * h + 40, 64 * h:64 * h + 64] = TiI.T
    # shift matrices: S7[xin, di, xout] = 1 iff xout == xin - (di-3)
    s7 = np.zeros((128, 7, 128), np.float32)
    for di in range(7):
        d = di - 3
        for xin in range(128):
            xo = xin - d
            if 0 <= xo < 128:
                s7[xin, di, xo] = 1.0
    s3m = np.ascontiguousarray(s7[:, 2:5, :])
    tau79 = np.zeros((7, 9), np.float32)
    for di in range(7):
        for k in range(9):
            t = (di - 3) - (k % 3) + 1
            tau79[di, k] = t if abs(t) <= 2 else 99.0
    owt = np.ascontiguousarray(OW.transpose(2, 1, 0))  # [49, 9, 18]

    def b16(a):
        return np.ascontiguousarray(a).astype(_bf16)

    return dict(
        fold_ln=fold_ln,
        a4=b16(a4), owt=b16(owt),
        tf2Re=b16(tf2Re), tf2Im=b16(tf2Im),
        ti2Re=b16(ti2Re), ti2Im=b16(ti2Im),
        identb=b16(np.eye(128, dtype=np.float32)),
        s7=b16(s7), s3m=b16(s3m),
        ow1T=b16(inputs['off_w1'][:, :, 0, 0].T.astype(np.float32)),
        tau3=b16(np.tile(np.array([-1, 0, 1], np.float32), (128, 1))),
        tau5=b16(np.tile(np.arange(-2, 3, dtype=np.float32), (128, 1))),
        tau79y=b16(np.tile(tau79.reshape(1, 7, 9, 1), (128, 1, 1, BND))),
        lnw=b16(np.tile(inputs['ln_w'].astype(np.float32), (128, 1))),
        lnb=b16(np.tile(inputs['ln_b'].astype(np.float32), (128, 1))),
        wo50=b16(np.concatenate(
            [(inputs['w3'][:, :, 0, 0] * lnscale[None, :]).T,
             (inputs['off_w3'][:, :, 0, 0] * lnscale[None, :]).T],
            axis=1).astype(np.float32)),
        b3r=np.tile(inputs['b3'].astype(np.float32)[None, :], (128, 1)),
    )


def make_in_maps(inputs):
    C = _hat_consts(inputs)
    x = np.asarray(inputs['x'][0], np.float32)
    xp = np.zeros((48, H + 8, W), np.float32)
    xp[:, 4:4 + H, :] = x
    xpb = xp.astype(_bf16)
    in_a = []
    for ci in range(NCORES):
        onesp = np.zeros((128, NYS), np.float32)
        for j in range(NYS):
            if 0 <= 16 * ci - 3 + j < H:
                onesp[:, j] = 1.0
        in_a.append(dict(
            xh=np.ascontiguousarray(xpb[:, 16 * ci + 1:16 * ci + 1 + NYS, :]),
            xt=np.ascontiguousarray(
                xpb[:, 16 * ci:16 * ci + NYX, :].transpose(2, 0, 1)),
            onesp=onesp.astype(_bf16),
            identb=C['identb'], ow1T=C['ow1T'], owt=C['owt'], a4=C['a4'],
            tf2Re=C['tf2Re'], tf2Im=C['tf2Im'], ti2Re=C['ti2Re'],
            ti2Im=C['ti2Im'], s7=C['s7'], tau3=C['tau3'], tau5=C['tau5'],
            tau79y=C['tau79y'], lnw=C['lnw'], lnb=C['lnb']))
    return C, in_a


def unscramble_vo(res_a):
    """[128, 16, 96] pixel-major per core -> [96, H, 128] channel-major."""
    parts = []
    for r in res_a:
        vp = r['vo_out'].reshape(2, 8, 8, 16, 96)   # [ph, py, px, pc, c]
        parts.append(vp.transpose(4, 0, 1, 3, 2).reshape(96, BND, 128))
    return np.concatenate(parts, axis=1)


def make_in_maps_b(C, vo_full):
    vop = np.zeros((96, H + 2, 128), _bf16)
    vop[:, 1:1 + H, :] = vo_full
    in_b = []
    for ci in range(NCORES):
        in_b.append(dict(
            voh=np.ascontiguousarray(vop[:, 16 * ci:16 * ci + NYV, :]),
            wo50=C['wo50'], b3r=C['b3r'], tau3=C['tau3'], s3m=C['s3m']))
    return in_b


_CACHE = {}


def kernel(**inputs):
    C, in_a = make_in_maps(inputs)
    key = 'nca_fold' if C['fold_ln'] else 'nca'
    if key not in _CACHE:
        _CACHE[key] = build_kernel_a(fold_ln=C['fold_ln'])
        _CACHE.setdefault('ncb', build_kernel_b())
    nca, ncb = _CACHE[key], _CACHE['ncb']
    res_a = bass_utils.run_bass_kernel_spmd(nca, in_a, core_ids=list(range(NCORES)))
    vo_full = unscramble_vo(res_a.results)
    in_b = make_in_maps_b(C, vo_full)
    res_b = bass_utils.run_bass_kernel_spmd(ncb, in_b, core_ids=list(range(NCORES)))
    # outp [128, 48, 16] x-major -> [48, 16, 128]
    out = np.concatenate([r['outp'].transpose(1, 2, 0) for r in res_b.results],
                         axis=1)
    return out[None].astype(np.float32)


# revision 59
# speedup vs baseline: 1.8278x; 1.0051x over previous
"""Trainium2 Bass kernel for nn_DeformAttn (deformable attention, patch-FFT).

Self-contained: hardcodes shapes for x [1,48,128,128], 8 NeuronCores,
y-band split (16 rows/core).

Design (per core):
- All deformable convs use dense hat-tap sampling with COLUMN-SHIFTED weight
  evaluation: for each x-shift delta, the 2D hat weights (hy*hx, both read
  from the delta-shifted offset field so they are evaluated at the OUTPUT
  pixel) premultiply the unshifted source on DVE (a slice of the work goes
  to the GpSimd/Pool engine); the x-shift-and-add runs on the PE as banded
  0/1 shift matmuls accumulating in PSUM, with the y-tap reduction riding
  the same PSUM accumulation (one matmul per ty plane).
- Layer-1 (1x1 deform, |off|<1): 3x3 taps.  Layer-2 (3x3 depthwise deform,
  |off|<2): 5x5 taps per kernel point, 7 global x-shifts.  Layer-3: 3x3 taps.
- qkv = A^T u with host-precomputed Khatri-Rao A (w2 (x) W1), bias via
  indicator/ones rows.  off2 via OW matrix on channel-major s1x.
- Hat weights are built on the UNSHIFTED offset field (one abs/relu pass)
  and then column-shifted by SBUF->SBUF DMAs (hy) / PE shift-matmuls
  (off1/off3); per-delta hat products chase the shift DMAs.
- x arrives in BOTH channel-major and x-major layouts from the host (no
  on-device input transposes).  vo leaves pixel-major; the host transposes.
- Patch FFT (8x8 circular conv): patch-PAIR block-diagonal real-DFT matmuls
  (128 partitions = 2 patches), complex pointwise on DVE, block-diag inverse.
- Everything bf16 on-chip except PSUM accumulation and LN statistics.
"""
import numpy as np
import ml_dtypes
_bf16 = np.float16
from contextlib import ExitStack

import concourse.bacc as bacc
import concourse.mybir as mybir
import concourse.tile as tile
from concourse.bass import AP
from concourse import bass_utils

dt = mybir.dt
F32 = dt.float32
BF16 = dt.float16
ALU = mybir.AluOpType
ACTF = mybir.ActivationFunctionType
AX = mybir.AxisListType

H = W = 128
BND = 16          # band rows per core
NYS = 22          # s1x rows per core (band +/-3)
NYX = 24          # x rows per core (band +/-4)
NYV = 18          # vo rows per core (band +/-1)
NCORES = 8
CB = [0, 128, 256, 314]   # uT K-chunk bases over 442 slots

def _v(t, off, dims):
    """View of tile t: keep its full partition dim, custom free dims."""
    return AP(t.tensor, t.offset + off, [list(t.ap[0])] + [list(d) for d in dims])


def _vp(t, p0, np_, off, dims):
    """View with partition sub-range [p0, p0+np_) and custom free dims."""
    st = t.ap[0][0]
    return AP(t.tensor, t.offset + p0 * st + off,
              [[st, np_]] + [list(d) for d in dims])


def build_kernel_a(fold_ln=False):
    nc = bacc.Bacc("TRN2", target_bir_lowering=False, debug=False)
    xh_d = nc.dram_tensor("xh", [48, NYS, W], BF16, kind="ExternalInput")
    xt_d = nc.dram_tensor("xt", [128, 48, NYX], BF16, kind="ExternalInput")
    onesp_d = nc.dram_tensor("onesp", [128, NYS], BF16, kind="ExternalInput")
    identb_d = nc.dram_tensor("identb", [128, 128], BF16, kind="ExternalInput")
    ow1T_d = nc.dram_tensor("ow1T", [48, 2], BF16, kind="ExternalInput")
    owt_d = nc.dram_tensor("owt", [49, 9, 18], BF16, kind="ExternalInput")
    a4_d = nc.dram_tensor("a4", [128, 4, 288], BF16, kind="ExternalInput")
    tf2Re_d = nc.dram_tensor("tf2Re", [128, 80], BF16, kind="ExternalInput")
    tf2Im_d = nc.dram_tensor("tf2Im", [128, 80], BF16, kind="ExternalInput")
    ti2Re_d = nc.dram_tensor("ti2Re", [80, 128], BF16, kind="ExternalInput")
    ti2Im_d = nc.dram_tensor("ti2Im", [80, 128], BF16, kind="ExternalInput")
    s7_d = nc.dram_tensor("s7", [128, 7, 128], BF16, kind="ExternalInput")
    tau3_d = nc.dram_tensor("tau3", [128, 3], BF16, kind="ExternalInput")
    tau5_d = nc.dram_tensor("tau5", [128, 5], BF16, kind="ExternalInput")
    tau79y_d = nc.dram_tensor("tau79y", [128, 7, 9, BND], BF16,
                              kind="ExternalInput")
    lnw_d = nc.dram_tensor("lnw", [128, 96], BF16, kind="ExternalInput")
    lnb_d = nc.dram_tensor("lnb", [128, 96], BF16, kind="ExternalInput")
    vo_out = nc.dram_tensor("vo_out", [128, BND, 96], BF16,
                            kind="ExternalOutput")

    with tile.TileContext(nc) as tc, ExitStack() as top:
        cpool = top.enter_context(tc.tile_pool(name="consts", bufs=1))
        xh = cpool.tile([48, NYS, W], BF16)
        xt = cpool.tile([128, 48, NYX], BF16)
        identb = cpool.tile([128, 128], BF16)
        ow1T = cpool.tile([48, 2], BF16)
        owt = cpool.tile([49, 9, 18], BF16)
        a4 = cpool.tile([128, 4, 288], BF16)
        tf2Re = cpool.tile([128, 80], BF16)
        tf2Im = cpool.tile([128, 80], BF16)
        ti2Re = cpool.tile([80, 128], BF16)
        ti2Im = cpool.tile([80, 128], BF16)
        s7 = cpool.tile([128, 7, 128], BF16)
        tau3 = cpool.tile([128, 3], BF16)
        tau5 = cpool.tile([128, 5], BF16)
        tau79y = cpool.tile([128, 7, 9, BND], BF16)
        lnw = cpool.tile([128, 96], BF16)
        lnb = cpool.tile([128, 96], BF16)
        onesp = cpool.tile([128, NYS], BF16, name="onesp")
        # one queue (the model serializes HWDGE anyway; issuing from Act
        # stalls the Act sequencer), ordered by first use
        first = [(xh, xh_d), (ow1T, ow1T_d), (identb, identb_d),
                 (s7, s7_d), (xt, xt_d),
                 (tau3, tau3_d), (onesp, onesp_d),
                 (owt, owt_d), (tau5, tau5_d), (tau79y, tau79y_d),
                 (a4, a4_d), (tf2Re, tf2Re_d), (tf2Im, tf2Im_d),
                 (ti2Re, ti2Re_d), (ti2Im, ti2Im_d)]
        if not fold_ln:
            first += [(lnw, lnw_d), (lnb, lnb_d)]
        for sb, dr in first:
            nc.sync.dma_start(sb[:], dr[:])

        scr = cpool.tile([1, 8], F32, name="scr")
        nc.vector.memset(scr[:], 1.0)
        nc.scalar.activation(scr[:], scr[:], ACTF.Sqrt)

        psum = top.enter_context(tc.tile_pool(name="psum", bufs=3, space="PSUM"))
        pupool = top.enter_context(tc.tile_pool(name="pu", bufs=4, space="PSUM"))

        big_cm = tc.tile_pool(name="big", bufs=1)
        bpool = big_cm.__enter__()
        s1x = bpool.tile([128, 49, NYS], BF16)
        u = bpool.tile([128, 442, BND], BF16)
        uT = bpool.tile([128, 4, 16, 128], BF16)
        qkv = bpool.tile([128, 16, 288], BF16)

        # ================= phase X: off1, layer-1 ==========================
        with tc.tile_pool(name="px", bufs=1) as p1:
            # off1 on s1x rows -> one psum, region-accumulated
            po = pupool.tile([128, 512], F32, tag="acc", name="po", bufs=3)
            for y in range(NYS):
                nc.tensor.matmul(po[:128, 2 * y:2 * y + 2],
                                 _v(xh, y * W, [[1, 128]]),
                                 ow1T[:], start=True, stop=True)
            off1pm = p1.tile([128, NYS, 2], BF16)
            nc.scalar.copy(off1pm[:], po[:128, :44])
            # shifted offset copies via PE shift-matmuls (edges auto-zero):
            # slot d=-1 (di=0): w[xin] = off[xin+1]; slot d=+1: off[xin-1]
            off1s3 = p1.tile([128, 3, NYS, 2], BF16)
            nc.vector.tensor_copy(off1s3[:, 1], off1pm[:])
            po1s = pupool.tile([128, 512], F32, tag="acc", name="po1s", bufs=3)
            nc.tensor.matmul(po1s[:128, 0:44], _v(s7, 4 * 128, [[1, 128]]),
                             _v(off1pm, 0, [[1, 44]]), start=True, stop=True)
            nc.tensor.matmul(po1s[:128, 44:88], _v(s7, 2 * 128, [[1, 128]]),
                             _v(off1pm, 0, [[1, 44]]), start=True, stop=True)
            nc.scalar.copy(_v(off1s3, 0, [[2 * 2 * NYS, 2], [1, 44]]),
                           po1s[:128, :88])
            # warm the PE p-state while W1d/tmp1 are being built so the
            # layer-1 matmuls run at full clock (junk transposes, never read)
            jp = psum.tile([128, 1024], BF16, tag="psb", name="jp", bufs=3)
            for _ in range(38):
                nc.tensor.transpose(jp[:128, :128], identb[:, :], identb[:, :])
            # W1d [128, 3d, 3ty, 22] = hat(oy_sh - (ty-1)) * hat(ox_sh - d)
            w1d = p1.tile([128, 3, 3, NYS], BF16)
            hx1 = p1.tile([128, 3, NYS], BF16)
            with nc.allow_low_precision(reason="hat weights bf16"):
                nc.vector.tensor_tensor(
                    out=w1d[:],
                    in0=_v(off1s3, 0, [[2 * NYS, 3], [0, 3], [2, NYS]]),
                    in1=_v(tau3, 0, [[0, 3], [1, 3], [0, NYS]]),
                    op=ALU.subtract)
                nc.scalar.activation(w1d[:], w1d[:], ACTF.Abs)
                nc.scalar.activation(w1d[:], w1d[:], ACTF.Relu,
                                     bias=1.0, scale=-1.0)
                nc.vector.tensor_tensor(
                    out=hx1[:],
                    in0=_v(off1s3, 1, [[2 * NYS, 3], [2, NYS]]),
                    in1=_v(tau3, 0, [[1, 3], [0, NYS]]),
                    op=ALU.subtract)
                nc.scalar.activation(hx1[:], hx1[:], ACTF.Abs)
                nc.scalar.activation(hx1[:], hx1[:], ACTF.Relu,
                                     bias=1.0, scale=-1.0)
                nc.vector.tensor_tensor(
                    out=w1d[:], in0=w1d[:],
                    in1=_v(hx1, 0, [[NYS, 3], [0, 3], [1, NYS]]),
                    op=ALU.mult)
            # layer-1 sampling: premult per delta, PE shift-accumulate
            ps1 = [pupool.tile([128, 512], F32, tag="acc", bufs=3, name=f"ps1_{c}")
                   for c in range(3)]
            tmp1s = []
            for di in range(3):
                tmp1 = p1.tile([128, 3, 48, NYS], BF16, tag="tmp1",
                               name="tmp1", bufs=3)
                with nc.allow_low_precision(reason="sampling taps bf16"):
                    nc.vector.tensor_tensor(
                        out=tmp1[:],
                        in0=_v(xt, 0, [[1, 3], [NYX, 48], [1, NYS]]),
                        in1=_v(w1d, di * 3 * NYS, [[NYS, 3], [0, 48], [1, NYS]]),
                        op=ALU.mult)
                tmp1s.append(tmp1)
            for di in range(3):
                tmp1 = tmp1s[di]
                lhs = _v(s7, (di + 2) * 128, [[1, 128]])  # delta=-1,0,1 -> slots 2,3,4
                for ty in range(3):
                    for ch in range(3):
                        nc.tensor.matmul(
                            ps1[ch][:128, :352], lhs,
                            _v(tmp1, ty * 48 * NYS + ch * 352, [[1, 352]]),
                            start=(di == 0 and ty == 0),
                            stop=(di == 2 and ty == 2))
            for ch in range(3):
                (nc.scalar.copy if ch != 1 else nc.vector.tensor_copy)(
                    _v(s1x, ch * 352, [[1, 352]]), ps1[ch][:128, :352])
            nc.sync.dma_start(_v(s1x, 48 * NYS, [[1, NYS]]), onesp[:])

        # ================= phase O: s1xT, off2, W2d =========================
        w2ds = [bpool.tile([128, 9, 5, BND], BF16, name=f"w2d{di}")
                for di in range(7)]
        with tc.tile_pool(name="po2", bufs=1) as p2:
            hox7 = p2.tile([128, 7, 864], BF16)
            nc.gpsimd.memset(hox7[:], 0.0)
            s1xT = p2.tile([49, NYS, 130], BF16)
            nc.gpsimd.memset(_v(s1xT, 0, [[130, NYS], [1, 1]]), 0.0)
            nc.gpsimd.memset(_v(s1xT, 129, [[130, NYS], [1, 1]]), 0.0)
            for gi, (g0, gn) in enumerate([(0, 8), (8, 8), (16, 6)]):
                ps = psum.tile([128, 1024], BF16, tag="psb", name="ps", bufs=3)
                for i in range(gn):
                    nc.tensor.transpose(ps[:49, i * 128:(i + 1) * 128],
                                        _v(s1x, g0 + i, [[NYS, 49]]),
                                        identb[:, :])
                dst = _v(s1xT, g0 * 130 + 1, [[130, gn], [1, 128]])
                (nc.scalar.copy if gi % 2 == 0 else nc.vector.tensor_copy)(
                    dst, ps[:49, :gn * 128])
            # off2: per band row b, 9 taps accumulate; 2 psum region-tiles
            pofs = [pupool.tile([128, 512], F32, tag="acc", bufs=3, name=f"po2_{h}")
                    for h in range(2)]
            for b in range(BND):
                po2 = pofs[b // 8]
                col = 18 * (b % 8)
                for t in range(9):
                    ty, tx = divmod(t, 3)
                    nc.tensor.matmul(
                        po2[:128, col:col + 18],
                        _v(s1xT, (b + 2 + ty) * 130 + tx, [[1, 128]]),
                        owt[:, t], start=(t == 0), stop=(t == 8))
            off2pm = p2.tile([128, BND, 18], BF16)
            nc.scalar.copy(_v(off2pm, 0, [[1, 144]]), pofs[0][:128, :144])
            nc.scalar.copy(_v(off2pm, 144, [[1, 144]]), pofs[1][:128, :144])
            # base hat_y + compact ox on the UNSHIFTED field, side by side:
            # hob = [hyb (720) | oxb (144)]
            hob = p2.tile([128, 864], BF16)
            with nc.allow_low_precision(reason="hat weights bf16"):
                nc.vector.tensor_tensor(
                    out=_v(hob, 0, [[80, 9], [16, 5], [1, BND]]),
                    in0=_v(off2pm, 0, [[2, 9], [0, 5], [18, BND]]),
                    in1=_v(tau5, 0, [[0, 9], [1, 5], [0, BND]]),
                    op=ALU.subtract)
                nc.vector.tensor_copy(
                    _v(hob, 720, [[16, 9], [1, BND]]),
                    _v(off2pm, 1, [[2, 9], [18, BND]]))
                nc.scalar.activation(_v(hob, 0, [[1, 720]]),
                                     _v(hob, 0, [[1, 720]]), ACTF.Abs)
                nc.scalar.activation(_v(hob, 0, [[1, 720]]),
                                     _v(hob, 0, [[1, 720]]), ACTF.Relu,
                                     bias=1.0, scale=-1.0)
            # column-shifted copies hox7[xin, di] = hob[xin - d] via DMAs;
            # edge partitions stay zero from the early Pool memset.  hxb and
            # the w2d slice for each di chase its shift DMA.
            hxb = p2.tile([128, 7, 9, BND], BF16)
            for d in range(-3, 4):
                di = d + 3
                if d > 0:
                    nc.sync.dma_start(hox7[d:128, di], hob[0:128 - d])
                elif d < 0:
                    nc.sync.dma_start(hox7[0:128 + d, di], hob[-d:128])
                else:
                    nc.sync.dma_start(hox7[:, di], hob[:])
                hxd = _v(hxb, di * 9 * BND, [[BND, 9], [1, BND]])
                with nc.allow_low_precision(reason="hat weights bf16"):
                    nc.gpsimd.tensor_tensor(
                        out=hxd,
                        in0=_v(hox7, di * 864 + 720, [[16, 9], [1, BND]]),
                        in1=_v(tau79y, di * 9 * BND, [[BND, 9], [1, BND]]),
                        op=ALU.subtract)
                    nc.scalar.activation(hxd, hxd, ACTF.Abs)
                    nc.scalar.activation(hxd, hxd, ACTF.Relu,
                                         bias=1.0, scale=-1.0)
                    nc.vector.tensor_tensor(
                        out=w2ds[di][:],
                        in0=_v(hox7, di * 864, [[80, 9], [16, 5], [1, BND]]),
                        in1=_v(hxb, di * 9 * BND, [[BND, 9], [0, 5], [1, BND]]),
                        op=ALU.mult)

        # keep the PE p-state warm across the hox7 shift-DMA stall so the
        # first sampling groups run at full clock
        jp2 = psum.tile([128, 1024], BF16, tag="psb", name="jp2", bufs=3)
        for _ in range(48):
            nc.tensor.transpose(jp2[:128, :128], identb[:, :], identb[:, :])

        # ========== phase S: sampling (pipelined premults, uT dribbled) ====
        # Premults for group k+1 (DVE) / k+2 (Pool) issue while the PE
        # consumes group k's 50 stall-free matmuls; uT transposes fill the
        # inter-group gaps so the PE p-state stays ramped.
        DVE_S = [0, 1, 3, 4]
        MM_S = [0, 1, 3, 4, 2]

        def emit_uT_group(c, g, eng):
            # c 0..2: full 128-slot chunks; c 3: slots 378..441 (64; slots
            # 378..383 are zeroed in a4 since chunk 2 already counts them)
            base, nsl = (CB[c], 128) if c < 3 else (378, 64)
            ps = psum.tile([128, 1024], BF16, tag="psb", name="ps", bufs=3)
            for yy in range(8):
                y = g * 8 + yy
                nc.tensor.transpose(
                    ps[:nsl, yy * 128:(yy + 1) * 128],
                    _v(u, base * BND + y, [[BND, nsl]]),
                    identb[:, :])
            dst = _vp(uT, 0, nsl, c * 2048 + g * 64,
                      [[8, 8], [128, 16], [1, 8]])
            (nc.scalar.copy if eng == 0 else nc.vector.tensor_copy)(
                dst, ps[:nsl, :1024])

        with tc.tile_pool(name="psmp", bufs=1) as p3:
            tmps = {}

            def emit_pool_premult(k):
                ki, kj = divmod(k, 3)
                tmp = p3.tile([128, 5, 49, BND], BF16, tag="tmpp",
                              name="tmpp", bufs=3)
                with nc.allow_low_precision(reason="sampling taps bf16"):
                    nc.gpsimd.tensor_tensor(
                        out=tmp[:],
                        in0=_v(s1x, ki, [[1, 5], [NYS, 49], [1, BND]]),
                        in1=_v(w2ds[kj + 2], k * 5 * BND,
                               [[BND, 5], [0, 49], [1, BND]]),
                        op=ALU.mult)
                tmps[(k, 2)] = tmp

            def emit_dve_premults(k):
                ki, kj = divmod(k, 3)
                for s in DVE_S:
                    tmp = p3.tile([128, 5, 49, BND], BF16, tag="tmp",
                                  name="tmp", bufs=10)
                    with nc.allow_low_precision(reason="sampling taps bf16"):
                        nc.vector.tensor_tensor(
                            out=tmp[:],
                            in0=_v(s1x, ki, [[1, 5], [NYS, 49], [1, BND]]),
                            in1=_v(w2ds[kj + s], k * 5 * BND,
                                   [[BND, 5], [0, 49], [1, BND]]),
                            op=ALU.mult)
                    tmps[(k, s)] = tmp

            # (chunk, group) transposes dribbled after group k's matmuls
            UT_SCHED = {3: [(0, 0)], 4: [(0, 1)], 6: [(1, 0)], 7: [(1, 1)],
                        8: [(2, 0), (2, 1)]}

            emit_pool_premult(0)
            emit_pool_premult(1)
            emit_dve_premults(0)
            for k in range(9):
                ki, kj = divmod(k, 3)
                if k < 8:
                    emit_dve_premults(k + 1)
                if k < 7:
                    emit_pool_premult(k + 2)
                if k < 8:
                    for c, g in UT_SCHED.get(k, []):
                        emit_uT_group(c, g, 0)
                pk = [pupool.tile([128, 512], F32, tag="acc", bufs=3,
                                  name=f"pk{ch}") for ch in range(2)]
                mm_order = [0, 1, 2, 3, 4] if k == 8 else MM_S
                for si, s in enumerate(mm_order):
                    di = kj + s
                    tmp = tmps.pop((k, s))
                    lhs = _v(s7, di * 128, [[1, 128]])
                    for ty in range(5):
                        for ch in range(2):
                            nc.tensor.matmul(
                                pk[ch][:128, :392], lhs,
                                _v(tmp, ty * 784 + ch * 392, [[1, 392]]),
                                start=(si == 0 and ty == 0),
                                stop=(si == 4 and ty == 4))
                nc.scalar.copy(_v(u, k * 784, [[1, 392]]),
                               pk[0][:128, :392])
                (nc.vector.tensor_copy if k == 8 else nc.scalar.copy)(
                    _v(u, k * 784 + 392, [[1, 392]]), pk[1][:128, :392])
                if k == 8:
                    nc.vector.memset(_v(u, 441 * BND, [[1, BND]]), 1.0)
                    for ci, (c, g) in enumerate(UT_SCHED[8]):
                        emit_uT_group(c, g, ci % 2)

        # ================= phase Q + FFT ====================================
        with tc.tile_pool(name="pfft", bufs=1) as fp:
            qhRe = fp.tile([80, 16, 192], BF16)
            qhIm = fp.tile([80, 16, 192], BF16)
            for g in range(2):
                emit_uT_group(3, g, g % 2)
            for pc in range(16):
                qp = pupool.tile([128, 512], F32, tag="acc", bufs=3,
                                 name="qp")
                for c in range(3):
                    nc.tensor.matmul(qp[:128, :288],
                                     _v(uT, c * 2048 + pc * 128, [[1, 128]]),
                                     a4[:, c], start=(c == 0), stop=False)
                nc.tensor.matmul(qp[:128, :288],
                                 _vp(uT, 0, 64, 3 * 2048 + pc * 128,
                                     [[1, 128]]),
                                 _vp(a4, 0, 64, 3 * 288, [[1, 288]]),
                                 start=False, stop=True)
                if pc % 2 == 0:
                    nc.scalar.copy(_v(qkv, pc * 288, [[1, 288]]),
                                   qp[:128, :288])
                else:
                    nc.vector.tensor_copy(_v(qkv, pc * 288, [[1, 288]]),
                                          qp[:128, :288])
                if pc % 2 == 1:
                    g = pc // 2
                    rhs = _v(qkv, 2 * g * 288, [[288, 2], [1, 192]])
                    psR = psum.tile([128, 512], F32, tag="ps", name="ps",
                                    bufs=2)
                    nc.tensor.matmul(psR[:80, :384], tf2Re[:], rhs,
                                     start=True, stop=True)
                    nc.scalar.copy(_v(qhRe, 2 * g * 192, [[1, 384]]),
                                   psR[:80, :384])
                    psI = psum.tile([128, 512], F32, tag="ps", name="ps",
                                    bufs=2)
                    nc.tensor.matmul(psI[:80, :384], tf2Im[:], rhs,
                                     start=True, stop=True)
                    (nc.vector.tensor_copy if g % 2 == 0 else nc.scalar.copy)(
                        _v(qhIm, 2 * g * 192, [[1, 384]]), psI[:80, :384])
            # complex pointwise + inverse DFT + LN partial stats, pipelined
            # per quarter (4 patch-pairs each)
            ohRe = fp.tile([80, 16, 96], BF16)
            ohIm = fp.tile([80, 16, 96], BF16)
            t1 = fp.tile([80, 16, 96], BF16)
            t2 = fp.tile([80, 16, 96], BF16)
            t1b = fp.tile([80, 16, 96], BF16)
            t2b = fp.tile([80, 16, 96], BF16)
            osb = fp.tile([128, 16, 96], BF16)
            ssum = fp.tile([128, BND], F32)
            sqs = fp.tile([128, BND], F32)
            sq = fp.tile([128, 16, 96], BF16)
            with nc.allow_low_precision(reason="fft products bf16"):
                for q in range(4):
                    o = q * 4 * 192
                    arq = _v(qhRe, o, [[192, 4], [1, 96]])
                    brq = _v(qhRe, o + 96, [[192, 4], [1, 96]])
                    aiq = _v(qhIm, o, [[192, 4], [1, 96]])
                    biq = _v(qhIm, o + 96, [[192, 4], [1, 96]])
                    oq = q * 384
                    t1q = _v(t1, oq, [[1, 384]])
                    t2q = _v(t2, oq, [[1, 384]])
                    t1bq = _v(t1b, oq, [[1, 384]])
                    t2bq = _v(t2b, oq, [[1, 384]])
                    nc.vector.tensor_tensor(out=t1q, in0=arq, in1=brq,
                                            op=ALU.mult)
                    nc.gpsimd.tensor_tensor(out=t2q, in0=aiq, in1=biq,
                                            op=ALU.mult)
                    nc.vector.tensor_tensor(out=_v(ohRe, oq, [[1, 384]]),
                                            in0=t1q, in1=t2q, op=ALU.subtract)
                    nc.gpsimd.tensor_tensor(out=t2bq, in0=aiq, in1=brq,
                                            op=ALU.mult)
                    nc.vector.tensor_tensor(out=t1bq, in0=arq, in1=biq,
                                            op=ALU.mult)
                    nc.vector.tensor_tensor(out=_v(ohIm, oq, [[1, 384]]),
                                            in0=t1bq, in1=t2bq, op=ALU.add)
                    ps = psum.tile([128, 512], F32, tag="ps", name="ps", bufs=2)
                    nc.tensor.matmul(ps[:128, :384], ti2Re[:],
                                     _v(ohRe, oq, [[1, 384]]),
                                     start=True, stop=False)
                    nc.tensor.matmul(ps[:128, :384], ti2Im[:],
                                     _v(ohIm, oq, [[1, 384]]),
                                     start=False, stop=True)
                    (nc.scalar.copy if q % 2 == 0 else nc.vector.tensor_copy)(
                        _v(osb, oq, [[1, 384]]), ps[:128, :384])
                    nc.vector.tensor_reduce(
                        out=_v(ssum, q * 4, [[1, 4]]),
                        in_=_v(osb, oq, [[96, 4], [1, 96]]),
                        axis=AX.X, op=ALU.add)
                    nc.scalar.activation(_v(sq, oq, [[1, 384]]),
                                         _v(osb, oq, [[1, 384]]), ACTF.Square)
                    nc.vector.tensor_reduce(
                        out=_v(sqs, q * 4, [[1, 4]]),
                        in_=_v(sq, oq, [[96, 4], [1, 96]]),
                        axis=AX.X, op=ALU.add)
            # LayerNorm stats -> normalize -> gate by v, half at a time
            mu = fp.tile([128, BND], F32)
            var = fp.tile([128, BND], F32)
            musq = fp.tile([128, BND], F32)
            mub = fp.tile([128, BND, 96], BF16)
            invb = fp.tile([128, BND, 96], BF16)
            on = fp.tile([128, 16, 96], BF16)
            vo = fp.tile([128, 16, 96], BF16)
            nc.scalar.activation(mu[:], ssum[:], ACTF.Copy, scale=1.0 / 96)
            nc.scalar.activation(var[:], sqs[:], ACTF.Copy, scale=1.0 / 96)
            nc.scalar.activation(musq[:], mu[:], ACTF.Square)
            nc.vector.tensor_tensor(out=var[:], in0=var[:], in1=musq[:],
                                    op=ALU.subtract)
            std = fp.tile([128, BND], F32)
            inv = fp.tile([128, BND], F32)
            nc.vector.tensor_scalar_add(out=var[:], in0=var[:], scalar1=1e-5)
            nc.scalar.activation(std[:], var[:], ACTF.Sqrt)
            nc.vector.reciprocal(inv[:], std[:])
            nc.gpsimd.tensor_copy(mub[:], _v(mu, 0, [[1, BND], [0, 96]]))
            nc.scalar.copy(invb[:], _v(inv, 0, [[1, BND], [0, 96]]))
            with nc.allow_low_precision(reason="ln bf16"):
                for h in range(2):
                    oh = h * 8 * 96
                    onh = _v(on, oh, [[1, 768]])
                    nc.vector.tensor_tensor(
                        out=onh, in0=_v(osb, oh, [[1, 768]]),
                        in1=_v(mub, oh, [[1, 768]]),
                        op=ALU.subtract)
                    nc.vector.tensor_tensor(
                        out=onh, in0=onh,
                        in1=_v(invb, oh, [[1, 768]]),
                        op=ALU.mult)
                    if not fold_ln:
                        nc.vector.tensor_tensor(
                            out=onh, in0=onh,
                            in1=_v(lnw, 0, [[0, 8], [1, 96]]), op=ALU.mult)
                        nc.vector.tensor_tensor(
                            out=onh, in0=onh,
                            in1=_v(lnb, 0, [[0, 8], [1, 96]]), op=ALU.add)
                    nc.vector.tensor_tensor(
                        out=_v(vo, oh, [[1, 768]]), in0=onh,
                        in1=_v(qkv, 192 + h * 8 * 288, [[288, 8], [1, 96]]),
                        op=ALU.mult)
                    nc.sync.dma_start(vo_out[:, 8 * h:8 * (h + 1)],
                                      _v(vo, oh, [[1, 768]]))
        big_cm.__exit__(None, None, None)
    nc.compile()
    return nc


def build_kernel_b():
    """Layer-3: out = W3 @ deform_sample(vo) + b3 == deform_sample(W3 @ vo) + b3
    (sampling offsets are channel-shared, so the contraction commutes).
    W3-contraction and off3 ride ONE fused per-row matmul; output leaves
    x-major (host transposes back)."""
    nc = bacc.Bacc("TRN2", target_bir_lowering=False, debug=False)
    voh_d = nc.dram_tensor("voh", [96, NYV, 128], BF16, kind="ExternalInput")
    wo50_d = nc.dram_tensor("wo50", [96, 50], BF16, kind="ExternalInput")
    b3r_d = nc.dram_tensor("b3r", [128, 48], F32, kind="ExternalInput")
    tau3_d = nc.dram_tensor("tau3", [128, 3], BF16, kind="ExternalInput")
    s3_d = nc.dram_tensor("s3m", [128, 3, 128], BF16, kind="ExternalInput")
    out_d = nc.dram_tensor("outp", [128, 48, BND], F32, kind="ExternalOutput")
    with tile.TileContext(nc) as tc, ExitStack() as top:
        pool = top.enter_context(tc.tile_pool(name="b", bufs=1))
        pup = top.enter_context(tc.tile_pool(name="pub", bufs=4, space="PSUM"))
        voh = pool.tile([96, NYV, 128], BF16)
        wo50 = pool.tile([96, 50], BF16)
        b3r = pool.tile([128, 48], F32)
        tau3 = pool.tile([128, 3], BF16)
        s3m = pool.tile([128, 3, 128], BF16)
        nc.sync.dma_start(wo50[:], wo50_d[:])
        for rc in range(3):
            nc.sync.dma_start(voh[:, 6 * rc:6 * (rc + 1)],
                              voh_d[:, 6 * rc:6 * (rc + 1)])
        for sb, dr in [(tau3, tau3_d), (s3m, s3_d), (b3r, b3r_d)]:
            nc.sync.dma_start(sb[:], dr[:])
        # fused [W3 | ow3] per-row contraction -> wvx2 [128, 50, 18] c-major
        wvx2 = pool.tile([128, 50, NYV], BF16)
        pws = [pup.tile([128, 512], F32, tag="acc", bufs=4, name=f"pw{h}")
               for h in range(2)]
        for y in range(NYV):
            nc.tensor.matmul(pws[y // 9][:128, (y % 9) * 50:(y % 9) * 50 + 50],
                             _v(voh, y * 128, [[1, 128]]),
                             wo50[:], start=True, stop=True)
        for h in range(2):
            dst = _v(wvx2, h * 9, [[1, 9], [NYV, 50]])
            (nc.scalar.copy if h == 0 else nc.vector.tensor_copy)(
                dst, pws[h][:128, :450])
        # off3 values live at wvx2 cols 48:50: off3[x, o, y] = wvx2[x, 48+o, y]
        off3s3 = pool.tile([128, 3, BND, 2], BF16)
        # center (d=0): off3s3[:, 1, b, o] = off3[x, o, b+1]
        nc.vector.tensor_copy(
            off3s3[:, 1], _v(wvx2, 48 * NYV + 1, [[1, BND], [NYV, 2]]))
        # shifted slots via PE shift-matmuls (edges auto-zero):
        # slot 0 (d=-1): off[xin+1] -> s3m slot 2; slot 2 (d=+1): slot 0
        p3s = pup.tile([128, 512], F32, tag="acc", name="p3s", bufs=4)
        nc.tensor.matmul(p3s[:128, 0:32], _v(s3m, 2 * 128, [[1, 128]]),
                         _v(off3s3, 32, [[1, 32]]), start=True, stop=True)
        nc.tensor.matmul(p3s[:128, 32:64], _v(s3m, 0, [[1, 128]]),
                         _v(off3s3, 32, [[1, 32]]), start=True, stop=True)
        nc.scalar.copy(_v(off3s3, 0, [[64, 2], [1, 32]]), p3s[:128, :64])
        # warm the PE p-state while the hat weights are built (s3m slot 1
        # is the identity matrix; junk transposes, never read)
        jb = pup.tile([128, 1024], BF16, tag="jnk", name="jb", bufs=1)
        for _ in range(34):
            nc.tensor.transpose(jb[:128, :128], _v(s3m, 128, [[1, 128]]),
                                _v(s3m, 128, [[1, 128]]))
        w3d = pool.tile([128, 3, 3, BND], BF16)
        hx3 = pool.tile([128, 3, BND], BF16)
        with nc.allow_low_precision(reason="hat weights bf16"):
            nc.vector.tensor_tensor(
                out=w3d[:],
                in0=_v(off3s3, 0, [[2 * BND, 3], [0, 3], [2, BND]]),
                in1=_v(tau3, 0, [[0, 3], [1, 3], [0, BND]]),
                op=ALU.subtract)
            nc.scalar.activation(w3d[:], w3d[:], ACTF.Abs)
            nc.scalar.activation(w3d[:], w3d[:], ACTF.Relu, bias=1.0, scale=-1.0)
            nc.vector.tensor_tensor(
                out=hx3[:],
                in0=_v(off3s3, 1, [[2 * BND, 3], [2, BND]]),
                in1=_v(tau3, 0, [[1, 3], [0, BND]]),
                op=ALU.subtract)
            nc.scalar.activation(hx3[:], hx3[:], ACTF.Abs)
            nc.scalar.activation(hx3[:], hx3[:], ACTF.Relu, bias=1.0, scale=-1.0)
            nc.vector.tensor_tensor(
                out=w3d[:], in0=w3d[:],
                in1=_v(hx3, 0, [[BND, 3], [0, 3], [1, BND]]),
                op=ALU.mult)
        pv = [pup.tile([128, 512], F32, tag="acc", bufs=4, name=f"pv{ch}")
              for ch in range(2)]
        for di in range(3):
            tmp3 = pool.tile([128, 3, 48, BND], BF16, tag="tmp3",
                             name="tmp3", bufs=2)
            with nc.allow_low_precision(reason="sampling taps bf16"):
                nc.vector.tensor_tensor(
                    out=tmp3[:, 0:2],
                    in0=_v(wvx2, 0, [[1, 2], [NYV, 48], [1, BND]]),
                    in1=_v(w3d, di * 3 * BND, [[BND, 2], [0, 48], [1, BND]]),
                    op=ALU.mult)
                nc.vector.tensor_tensor(
                    out=tmp3[:, 2],
                    in0=_v(wvx2, 2, [[NYV, 48], [1, BND]]),
                    in1=_v(w3d, di * 3 * BND + 2 * BND, [[0, 48], [1, BND]]),
                    op=ALU.mult)
            lhs = _v(s3m, di * 128, [[1, 128]])
            for ty in range(3):
                for ch in range(2):
                    nc.tensor.matmul(
                        pv[ch][:128, :384], lhs,
                        _v(tmp3, ty * 48 * BND + ch * 384, [[1, 384]]),
                        start=(di == 0 and ty == 0),
                        stop=(di == 2 and ty == 2))
        # evict + bias -> x-major f32 out [128, 48, 16]
        s3of = pool.tile([128, 48, BND], F32)
        for ch in range(2):
            nc.vector.tensor_tensor(
                out=_v(s3of, ch * 384, [[1, 384]]),
                in0=pv[ch][:128, :384],
                in1=_vp(b3r, 0, 128, ch * 24, [[1, 24], [0, BND]]),
                op=ALU.add)
            nc.sync.dma_start(out_d[:, ch * 24:(ch + 1) * 24],
                              _v(s3of, ch * 384, [[1, 384]]))
    nc.compile()
    return nc


def _hat_consts(inputs):
    fold_ln = not np.any(inputs['ln_b'])
    lnscale = (inputs['ln_w'].astype(np.float32) if fold_ln
               else np.ones(96, np.float32))
    w1 = inputs['w1'][:, :, 0, 0].astype(np.float32)
    b1 = inputs['b1'].astype(np.float32)
    ow2 = inputs['off_w2'].astype(np.float32)
    w2f = inputs['w2'][:, 0].reshape(288, 9).astype(np.float32)
    OW = np.zeros((18, 9, 49), np.float32)
    for ty in range(3):
        for tx in range(3):
            t = ty * 3 + tx
            OW[:, t, :48] = ow2[:, :, ty, tx] @ w1
            OW[:, t, 48] = ow2[:, :, ty, tx] @ b1
    A = np.zeros((288, 9, 49), np.float32)
    for k in range(9):
        A[:, k, :48] = w2f[:, k:k + 1] * w1
        A[:, k, 48] = w2f[:, k] * b1
    Afull = np.zeros((442, 288), np.float32)
    Afull[:441] = A.reshape(288, 441).T
    Afull[441] = inputs['b2'].astype(np.float32)
    a4 = np.zeros((128, 4, 288), np.float32)
    for c in range(3):
        a4[:, c, :] = Afull[CB[c]:CB[c] + 128]
    a4[:64, 3, :] = Afull[378:442]
    a4[:6, 3, :] = 0.0  # slots 378..383 already counted in chunk 2
    # block-diag pair DFT matrices
    TfRe = np.zeros((40, 64), np.float32)
    TfIm = np.zeros((40, 64), np.float32)
    basis = np.zeros((8, 8))
    for y in range(8):
        for x in range(8):
            basis[:] = 0.0
            basis[y, x] = 1.0
            Fz = np.fft.rfft2(basis)
            TfRe[:, y * 8 + x] = Fz.real.reshape(-1)
            TfIm[:, y * 8 + x] = Fz.imag.reshape(-1)
    TiR = np.zeros((64, 40), np.float32)
    TiI = np.zeros((64, 40), np.float32)
    for j in range(40):
        fy, fx = divmod(j, 5)
        Z = np.zeros((8, 5), np.complex64)
        Z[fy, fx] = 1.0
        TiR[:, j] = np.fft.irfft2(Z, s=(8, 8)).reshape(-1)
        Z[fy, fx] = 1j
        TiI[:, j] = np.fft.irfft2(Z, s=(8, 8)).reshape(-1)
    tf2Re = np.zeros((128, 80), np.float32)
    tf2Im = np.zeros((128, 80), np.float32)
    ti2Re = np.zeros((80, 128), np.float32)
    ti2Im = np.zeros((80, 128), np.float32)
    for h in range(2):
        tf2Re[64 * h:64 * h + 64, 40 * h:40 * h + 40] = TfRe.T
        tf2Im[64 * h:64 * h + 64, 40 * h:40 * h + 40] = TfIm.T
        ti2Re[40 * h:40 * h + 40, 64 * h:64 * h + 64] = TiR.T
        ti2Im[40 * h:40 * h + 40, 64 * h:64 * h + 64] = TiI.T
    # shift matrices: S7[xin, di, xout] = 1 iff xout == xin - (di-3)
    s7 = np.zeros((128, 7, 128), np.float32)
    for di in range(7):
        d = di - 3
        for xin in range(128):
            xo = xin - d
            if 0 <= xo < 128:
                s7[xin, di, xo] = 1.0
    s3m = np.ascontiguousarray(s7[:, 2:5, :])
    tau79 = np.zeros((7, 9), np.float32)
    for di in range(7):
        for k in range(9):
            t = (di - 3) - (k % 3) + 1
            tau79[di, k] = t if abs(t) <= 2 else 99.0
    owt = np.ascontiguousarray(OW.transpose(2, 1, 0))  # [49, 9, 18]

    def b16(a):
        return np.ascontiguousarray(a).astype(_bf16)

    return dict(
        fold_ln=fold_ln,
        a4=b16(a4), owt=b16(owt),
        tf2Re=b16(tf2Re), tf2Im=b16(tf2Im),
        ti2Re=b16(ti2Re), ti2Im=b16(ti2Im),
        identb=b16(np.eye(128, dtype=np.float32)),
        s7=b16(s7), s3m=b16(s3m),
        ow1T=b16(inputs['off_w1'][:, :, 0, 0].T.astype(np.float32)),
        tau3=b16(np.tile(np.array([-1, 0, 1], np.float32), (128, 1))),
        tau5=b16(np.tile(np.arange(-2, 3, dtype=np.float32), (128, 1))),
        tau79y=b16(np.tile(tau79.reshape(1, 7, 9, 1), (128, 1, 1, BND))),
        lnw=b16(np.tile(inputs['ln_w'].astype(np.float32), (128, 1))),
        lnb=b16(np.tile(inputs['ln_b'].astype(np.float32), (128, 1))),
        wo50=b16(np.concatenate(
            [(inputs['w3'][:, :, 0, 0] * lnscale[None, :]).T,
             (inputs['off_w3'][:, :, 0, 0] * lnscale[None, :]).T],
            axis=1).astype(np.float32)),
        b3r=np.tile(inputs['b3'].astype(np.float32)[None, :], (128, 1)),
    )


def make_in_maps(inputs):
    C = _hat_consts(inputs)
    x = np.asarray(inputs['x'][0], np.float32)
    xp = np.zeros((48, H + 8, W), np.float32)
    xp[:, 4:4 + H, :] = x
    xpb = xp.astype(_bf16)
    in_a = []
    for ci in range(NCORES):
        onesp = np.zeros((128, NYS), np.float32)
        for j in range(NYS):
            if 0 <= 16 * ci - 3 + j < H:
                onesp[:, j] = 1.0
        in_a.append(dict(
            xh=np.ascontiguousarray(xpb[:, 16 * ci + 1:16 * ci + 1 + NYS, :]),
            xt=np.ascontiguousarray(
                xpb[:, 16 * ci:16 * ci + NYX, :].transpose(2, 0, 1)),
            onesp=onesp.astype(_bf16),
            identb=C['identb'], ow1T=C['ow1T'], owt=C['owt'], a4=C['a4'],
            tf2Re=C['tf2Re'], tf2Im=C['tf2Im'], ti2Re=C['ti2Re'],
            ti2Im=C['ti2Im'], s7=C['s7'], tau3=C['tau3'], tau5=C['tau5'],
            tau79y=C['tau79y'], lnw=C['lnw'], lnb=C['lnb']))
    return C, in_a


def unscramble_vo(res_a):
    """[128, 16, 96] pixel-major per core -> [96, H, 128] channel-major."""
    parts = []
    for r in res_a:
        vp = r['vo_out'].reshape(2, 8, 8, 16, 96)   # [ph, py, px, pc, c]
        parts.append(vp.transpose(4, 0, 1, 3, 2).reshape(96, BND, 128))
    return np.concatenate(parts, axis=1)


def make_in_maps_b(C, vo_full):
    vop = np.zeros((96, H + 2, 128), _bf16)
    vop[:, 1:1 + H, :] = vo_full
    in_b = []
    for ci in range(NCORES):
        in_b.append(dict(
            voh=np.ascontiguousarray(vop[:, 16 * ci:16 * ci + NYV, :]),
            wo50=C['wo50'], b3r=C['b3r'], tau3=C['tau3'], s3m=C['s3m']))
    return in_b


_CACHE = {}


def kernel(**inputs):
    C, in_a = make_in_maps(inputs)
    key = 'nca_fold' if C['fold_ln'] else 'nca'
    if key not in _CACHE:
        _CACHE[key] = build_kernel_a(fold_ln=C['fold_ln'])
        _CACHE.setdefault('ncb', build_kernel_b())
    nca, ncb = _CACHE[key], _CACHE['ncb']
    res_a = bass_utils.run_bass_kernel_spmd(nca, in_a, core_ids=list(range(NCORES)))
    vo_full = unscramble_vo(res_a.results)
    in_b = make_in_maps_b(C, vo_full)
    res_b = bass_utils.run_bass_kernel_spmd(ncb, in_b, core_ids=list(range(NCORES)))
    # outp [128, 48, 16] x-major -> [48, 16, 128]
    out = np.concatenate([r['outp'].transpose(1, 2, 0) for r in res_b.results],
                         axis=1)
    return out[None].astype(np.float32)
